# revision 14
# baseline (speedup 1.0000x reference)
# Trainium2 Bass kernel for AdvancedHybridHOIGNN (6x SAGEConv + 3x Conv1d + MHA + fuse).
#
# Sharding: 4096 nodes split 512/core across 8 cores. The SAGE neighbor
# aggregation is done as dense matmuls against a host-built normalized
# adjacency slice (A[dst, src]/deg, transposed, per core). Node features are
# replicated each layer via AllGather. The CNN branch needs a 128-node halo,
# gathered with indirect DMA using per-core index data so the single SPMD
# program stays uniform across cores. Attention is sequence-parallel over
# queries with AllGathered K/V.
#
# Layouts: activations flow in both node-major ([node_part, feat]) and
# feature-major ([feat_part, node]) forms; feature-major feeds matmul lhsT
# slices, node-major feeds LayerNorm (bn_stats reduces along free axis).

import sys

sys.path.insert(0, "/opt/trn_rl_repo")

import numpy as np

import concourse.bass as bass
import concourse.mybir as mybir
import concourse.tile as tile
from concourse import bacc
from concourse.bass_utils import run_bass_kernel_spmd
from concourse.masks import make_identity

FP = mybir.dt.float32
AF = mybir.ActivationFunctionType
ALU = mybir.AluOpType

N, H, OUT, L = 4096, 256, 64, 6
NCORES = 8
NPC = N // NCORES          # 512 nodes per core
NT = N // 128              # 32 node tiles globally
NTC = NPC // 128           # 4 node tiles per core
FT = H // 128              # 2 feature tiles
E2 = 2 * H                 # 512
HEADS, HD = 4, 128
HALO = 128                 # halo width for the conv branch (needs only 3)
CW = NPC + 2 * HALO        # 768: conv working width per core
EPS = 1e-5

_CACHE = {}


def _build(flags, dbg=False):
    """Trace + compile the SPMD Bass program. flags: which affine params are
    trivial (zeros/ones) and can be skipped."""
    nc = bacc.Bacc("TRN2", target_bir_lowering=False, debug=False,
                   num_devices=NCORES)
    RG = [list(range(NCORES))]

    # ---------------- kernel I/O ----------------
    x_full = nc.dram_tensor("x_full", [N, H], FP, kind="ExternalInput")
    x_own = nc.dram_tensor("x_own", [NPC, H], FP, kind="ExternalInput")
    a_ct = nc.dram_tensor("a_ct", [NT, 128, NPC], FP, kind="ExternalInput")
    wlT = nc.dram_tensor("wlT", [L, FT, 128, H], FP, kind="ExternalInput")
    wrT = nc.dram_tensor("wrT", [L, FT, 128, H], FP, kind="ExternalInput")
    cwT = nc.dram_tensor("cwT", [3, 3, FT, 128, H], FP, kind="ExternalInput")
    ipwT = nc.dram_tensor("ipwT", [4, 128, 3 * E2], FP, kind="ExternalInput")
    opwT = nc.dram_tensor("opwT", [4, 128, E2], FP, kind="ExternalInput")
    fwT = nc.dram_tensor("fwT", [4, 128, OUT], FP, kind="ExternalInput")
    idx_l = nc.dram_tensor("idx_l", [HALO, 1], mybir.dt.int32, kind="ExternalInput")
    idx_r = nc.dram_tensor("idx_r", [HALO, 1], mybir.dt.int32, kind="ExternalInput")
    mask_l = nc.dram_tensor("mask_l", [HALO, 1], FP, kind="ExternalInput")
    mask_r = nc.dram_tensor("mask_r", [HALO, 1], FP, kind="ExternalInput")
    # optional affine params (only staged when nontrivial)
    if not flags["bl0"]:
        bl_in = nc.dram_tensor("bl_in", [1, L, H], FP, kind="ExternalInput")
    if not flags["ln1"]:
        lng_in = nc.dram_tensor("lng_in", [1, L - 1, H], FP, kind="ExternalInput")
        lnb_in = nc.dram_tensor("lnb_in", [1, L - 1, H], FP, kind="ExternalInput")
    if not flags["cb0"]:
        cb_in = nc.dram_tensor("cb_in", [128, 3, FT, 1], FP, kind="ExternalInput")
    if not flags["cn1"]:
        cng_in = nc.dram_tensor("cng_in", [128, 3, FT, 1], FP, kind="ExternalInput")
        cnb_in = nc.dram_tensor("cnb_in", [128, 3, FT, 1], FP, kind="ExternalInput")
    if not flags["ipb0"]:
        ipb_in = nc.dram_tensor("ipb_in", [128, 12, 1], FP, kind="ExternalInput")
    if not flags["opb0"]:
        opb_in = nc.dram_tensor("opb_in", [128, 4, 1], FP, kind="ExternalInput")
    if not flags["an1"]:
        ang_in = nc.dram_tensor("ang_in", [128, 4, 1], FP, kind="ExternalInput")
        anb_in = nc.dram_tensor("anb_in", [128, 4, 1], FP, kind="ExternalInput")
    if not flags["fb0"]:
        fb_in = nc.dram_tensor("fb_in", [64, 1], FP, kind="ExternalInput")
    out_d = nc.dram_tensor("out", [NPC, OUT], FP, kind="ExternalOutput")
    if dbg:
        dbg_gnn = nc.dram_tensor("dbg_gnn", [NPC, H], FP, kind="ExternalOutput")
        dbg_cnn = nc.dram_tensor("dbg_cnn", [128, FT * CW], FP, kind="ExternalOutput")
        dbg_qkv = nc.dram_tensor("dbg_qkv", [128, 12 * NPC], FP, kind="ExternalOutput")
        dbg_ofm = nc.dram_tensor("dbg_ofm", [128, HEADS * NPC], FP, kind="ExternalOutput")
        dbg_opn = nc.dram_tensor("dbg_opn", [128, 4 * NPC], FP, kind="ExternalOutput")

    from contextlib import ExitStack

    with tile.TileContext(nc) as tc:
        stack = ExitStack()
        # long-lived pools on the right SBUF side; phase pools stack LIFO on the left
        singles = stack.enter_context(tc.tile_pool(name="singles", bufs=1, side="right"))
        dram = stack.enter_context(tc.tile_pool(name="dram", bufs=1, space="DRAM"))

        ident = singles.tile([128, 128], FP)
        make_identity(nc, ident[:])
        eps_t = singles.tile([128, 1], FP)
        nc.vector.memset(eps_t[:], EPS)
        ones_t = singles.tile([128, 1], FP)
        nc.vector.memset(ones_t[:], 1.0)
        ones_row = singles.tile([1, 128], FP)
        nc.vector.memset(ones_row[:], 1.0)

        # ---- phase 1: SAGE layers ----
        sagew = ExitStack()
        sw = sagew.enter_context(tc.tile_pool(name="sagew", bufs=1))
        act_sb = sw.tile([128, NT, NPC], FP)          # A_cT resident (64KB/part)
        for s in range(NT):
            nc.sync.dma_start(out=act_sb[:, s, :], in_=a_ct[s])
        wl_sb = sw.tile([128, L, FT, H], FP)
        wr_sb = sw.tile([128, L, FT, H], FP)
        for i in range(L):
            for ft in range(FT):
                nc.sync.dma_start(out=wl_sb[:, i, ft, :], in_=wlT[i, ft])
                nc.sync.dma_start(out=wr_sb[:, i, ft, :], in_=wrT[i, ft])
        if not flags["bl0"]:
            bl_sb = sw.tile([128, L, H], FP)
            nc.gpsimd.dma_start(out=bl_sb[:], in_=bl_in[:].to_broadcast([128, L, H]))
        if not flags["ln1"]:
            lng_sb = sw.tile([128, L - 1, H], FP)
            lnb_sb = sw.tile([128, L - 1, H], FP)
            nc.gpsimd.dma_start(out=lng_sb[:], in_=lng_in[:].to_broadcast([128, L - 1, H]))
            nc.gpsimd.dma_start(out=lnb_sb[:], in_=lnb_in[:].to_broadcast([128, L - 1, H]))

        ho_pool = stack.enter_context(tc.tile_pool(name="ho", bufs=2, side="right"))
        hs_pool = ExitStack()
        hstream = hs_pool.enter_context(tc.tile_pool(name="hstream", bufs=6))
        sage_ps = ExitStack()
        agg_ps = sage_ps.enter_context(tc.tile_pool(name="agg_ps", bufs=1, space="PSUM"))
        z_ps = sage_ps.enter_context(tc.tile_pool(name="z_ps", bufs=2, space="PSUM"))
        t_ps = sage_ps.enter_context(tc.tile_pool(name="t_ps", bufs=2, space="PSUM"))
        sage_tmp = ExitStack()
        stmp = sage_tmp.enter_context(tc.tile_pool(name="stmp", bufs=4))
        aggp = sage_tmp.enter_context(tc.tile_pool(name="aggsb", bufs=2))

        # initial own-slice: node-major + feature-major
        ho_nm = ho_pool.tile([128, NTC, H], FP, name="ho_nm0", tag="ho_nm")
        for nt in range(NTC):
            nc.sync.dma_start(out=ho_nm[:, nt, :], in_=x_own[nt * 128:(nt + 1) * 128, :])
        ho_fm = ho_pool.tile([128, FT, NPC], FP, name="ho_fm0", tag="ho_fm")
        for nt in range(NTC):
            for ft in range(FT):
                pt = t_ps.tile([128, 128], FP, tag="tps")
                nc.tensor.transpose(pt[:], ho_nm[:, nt, ft * 128:(ft + 1) * 128], ident[:])
                nc.vector.tensor_copy(out=ho_fm[:, ft, nt * 128:(nt + 1) * 128], in_=pt[:])

        cc_outs = []
        for i in range(L):
            hsrc = x_full if i == 0 else cc_outs[i - 1]
            # aggregation: agg_fm[f, d] = sum_s h[s, f] * A_cT[s, d]
            psa = [agg_ps.tile([128, NPC], FP, name=f"psa{i}_{ft}", tag=f"psa{ft}")
                   for ft in range(FT)]
            for s in range(NT):
                hk = hstream.tile([128, H], FP, tag="hk")
                nc.sync.dma_start(out=hk[:], in_=hsrc[s * 128:(s + 1) * 128, :])
                for ft in range(FT):
                    nc.tensor.matmul(psa[ft][:], hk[:, ft * 128:(ft + 1) * 128],
                                     act_sb[:, s, :], start=(s == 0), stop=(s == NT - 1))
            agg_fm = aggp.tile([128, FT, NPC], FP, tag="agg_fm")
            for ft in range(FT):
                nc.vector.tensor_copy(out=agg_fm[:, ft, :], in_=psa[ft][:])

            # z[n, o] = agg @ wl.T + h @ wr.T  (node-major out)
            ho_nm_new = ho_pool.tile([128, NTC, H], FP, name=f"ho_nm{i + 1}", tag="ho_nm")
            ho_fm_new = ho_pool.tile([128, FT, NPC], FP, name=f"ho_fm{i + 1}", tag="ho_fm")
            for nt in range(NTC):
                ns = slice(nt * 128, (nt + 1) * 128)
                psz = z_ps.tile([128, H], FP, tag="psz")
                for ft in range(FT):
                    nc.tensor.matmul(psz[:], agg_fm[:, ft, ns], wl_sb[:, i, ft, :],
                                     start=(ft == 0), stop=False)
                for ft in range(FT):
                    nc.tensor.matmul(psz[:], ho_fm[:, ft, ns], wr_sb[:, i, ft, :],
                                     start=False, stop=(ft == FT - 1))
                z_sb = stmp.tile([128, H], FP, tag="z_sb")
                if flags["bl0"]:
                    nc.vector.tensor_copy(out=z_sb[:], in_=psz[:])
                else:
                    nc.vector.tensor_tensor(out=z_sb[:], in0=psz[:],
                                            in1=bl_sb[:, i, :], op=ALU.add)
                if i < L - 1:
                    stat = stmp.tile([128, 6], FP, tag="stat")
                    nc.vector.bn_stats(out=stat[:], in_=z_sb[:])
                    mv = stmp.tile([128, 2], FP, tag="mv")
                    nc.vector.bn_aggr(out=mv[:], in_=stat[:])
                    sd = stmp.tile([128, 1], FP, tag="sd")
                    nc.scalar.activation(out=sd[:], in_=mv[:, 1:2], func=AF.Sqrt,
                                         bias=eps_t[:], scale=1.0)
                    nc.vector.reciprocal(out=sd[:], in_=sd[:])
                    zn = stmp.tile([128, H], FP, tag="zn")
                    nc.vector.tensor_scalar(out=zn[:], in0=z_sb[:], scalar1=mv[:, 0:1],
                                            scalar2=sd[:], op0=ALU.subtract, op1=ALU.mult)
                    if not flags["ln1"]:
                        nc.vector.tensor_tensor(out=zn[:], in0=zn[:],
                                                in1=lng_sb[:, i, :], op=ALU.mult)
                        nc.vector.tensor_tensor(out=zn[:], in0=zn[:],
                                                in1=lnb_sb[:, i, :], op=ALU.add)
                else:
                    zn = z_sb
                zr = stmp.tile([128, H], FP, tag="zr")
                nc.scalar.activation(out=zr[:], in_=zn[:], func=AF.Relu)
                nc.vector.tensor_add(out=ho_nm_new[:, nt, :], in0=zr[:], in1=ho_nm[:, nt, :])
                for ft in range(FT):
                    pt = t_ps.tile([128, 128], FP, tag="tps")
                    nc.tensor.transpose(pt[:], ho_nm_new[:, nt, ft * 128:(ft + 1) * 128], ident[:])
                    nc.vector.tensor_copy(out=ho_fm_new[:, ft, ns], in_=pt[:])
            ho_nm, ho_fm = ho_nm_new, ho_fm_new

            # AllGather the updated slice
            cc_in = dram.tile([NPC, H], FP, name=f"cc_in{i}")
            for nt in range(NTC):
                nc.sync.dma_start(out=cc_in[nt * 128:(nt + 1) * 128, :], in_=ho_nm[:, nt, :])
            cc_out = dram.tile([N, H], FP, name=f"cc_out{i}", addr_space="Shared")
            nc.gpsimd.collective_compute("AllGather", ALU.bypass, replica_groups=RG,
                                         ins=[cc_in.opt()], outs=[cc_out.opt()])
            cc_outs.append(cc_out)

        sage_tmp.close()
        hs_pool.close()
        sagew.close()
        sage_ps.close()
        gnn_full = cc_outs[L - 1]
        if dbg:
            for nt in range(NTC):
                nc.sync.dma_start(out=dbg_gnn[nt * 128:(nt + 1) * 128, :],
                                  in_=ho_nm[:, nt, :])

        # ---- phase 2: CNN branch (feature-major, nodes on free axis) ----
        cnn = ExitStack()
        cw_pool = cnn.enter_context(tc.tile_pool(name="cnnw", bufs=1))
        cfm_pool = cnn.enter_context(tc.tile_pool(name="cfm", bufs=2))
        ctmp = cnn.enter_context(tc.tile_pool(name="ctmp", bufs=1))
        cps = ExitStack()
        c_ps = cps.enter_context(tc.tile_pool(name="c_ps", bufs=2, space="PSUM"))
        s_ps = cps.enter_context(tc.tile_pool(name="s_ps", bufs=1, space="PSUM"))
        ctp_es = ExitStack()
        ct_ps = ctp_es.enter_context(tc.tile_pool(name="ct_ps", bufs=2, space="PSUM"))

        cw_sb = cw_pool.tile([128, 3, 3, FT, H], FP)
        for j in range(3):
            for k in range(3):
                for ft in range(FT):
                    nc.sync.dma_start(out=cw_sb[:, j, k, ft, :], in_=cwT[j, k, ft])
        if not flags["cb0"]:
            cb_sb = cw_pool.tile([128, 3, FT, 1], FP)
            nc.sync.dma_start(out=cb_sb[:], in_=cb_in[:])
        if not flags["cn1"]:
            cng_sb = cw_pool.tile([128, 3, FT, 1], FP)
            cnb_sb = cw_pool.tile([128, 3, FT, 1], FP)
            nc.sync.dma_start(out=cng_sb[:], in_=cng_in[:])
            nc.sync.dma_start(out=cnb_sb[:], in_=cnb_in[:])

        # window: [left halo | own 512 | right halo] node-major then transpose
        il_sb = cw_pool.tile([HALO, 1], mybir.dt.int32)
        ir_sb = cw_pool.tile([HALO, 1], mybir.dt.int32)
        ml_sb = cw_pool.tile([HALO, 1], FP)
        mr_sb = cw_pool.tile([HALO, 1], FP)
        nc.sync.dma_start(out=il_sb[:], in_=idx_l[:])
        nc.sync.dma_start(out=ir_sb[:], in_=idx_r[:])
        nc.sync.dma_start(out=ml_sb[:], in_=mask_l[:])
        nc.sync.dma_start(out=mr_sb[:], in_=mask_r[:])
        halo_l = ctmp.tile([HALO, H], FP, tag="halo")
        nc.gpsimd.indirect_dma_start(
            out=halo_l[:], out_offset=None, in_=gnn_full[:],
            in_offset=bass.IndirectOffsetOnAxis(ap=il_sb[:, :1], axis=0))
        nc.vector.tensor_scalar_mul(out=halo_l[:], in0=halo_l[:], scalar1=ml_sb[:])
        halo_r = ctmp.tile([HALO, H], FP, tag="halo")
        nc.gpsimd.indirect_dma_start(
            out=halo_r[:], out_offset=None, in_=gnn_full[:],
            in_offset=bass.IndirectOffsetOnAxis(ap=ir_sb[:, :1], axis=0))
        nc.vector.tensor_scalar_mul(out=halo_r[:], in0=halo_r[:], scalar1=mr_sb[:])

        c_fm = cfm_pool.tile([128, FT, CW], FP, tag="c_fm", name="c_fm_in")
        wnd = [halo_l[:]] + [ho_nm[:, nt, :] for nt in range(NTC)] + [halo_r[:]]
        for w, src in enumerate(wnd):
            for ft in range(FT):
                pt = ct_ps.tile([128, 128], FP, tag="ctps")
                nc.tensor.transpose(pt[:], src[:, ft * 128:(ft + 1) * 128], ident[:])
                nc.vector.tensor_copy(out=c_fm[:, ft, w * 128:(w + 1) * 128], in_=pt[:])
        ctp_es.close()

        # conv layers: compute output cols [1, CW-1)
        chunks = [(1, 512), (513, CW - 1 - 513)]
        for j in range(3):
            cr = cfm_pool.tile([128, FT, CW], FP, tag="c_fm", name=f"c_fm{j}")
            for ft in range(FT):  # guard stale edge cols
                nc.vector.memset(cr[:, ft, 0:1], 0.0)
                nc.vector.memset(cr[:, ft, CW - 1:CW], 0.0)
            for ot in range(FT):
                for (c0, cl) in chunks:
                    psc = c_ps.tile([128, 512], FP, tag="psc")
                    first = True
                    for k in range(3):
                        for it in range(FT):
                            nc.tensor.matmul(
                                psc[:, :cl],
                                cw_sb[:, j, k, it, ot * 128:(ot + 1) * 128],
                                cr_prev_slice(c_fm, it, c0 + k - 1, cl),
                                start=first, stop=(k == 2 and it == FT - 1))
                            first = False
                    if flags["cb0"]:
                        nc.scalar.activation(out=cr[:, ot, c0:c0 + cl], in_=psc[:, :cl],
                                             func=AF.Relu)
                    else:
                        nc.scalar.activation(out=cr[:, ot, c0:c0 + cl], in_=psc[:, :cl],
                                             func=AF.Relu, bias=cb_sb[:, j, ot, :], scale=1.0)
            # channel LayerNorm per node (partition reduce via ones-matmul)
            W = CW - 2
            sums = ctmp.tile([1, CW], FP, tag="sums")
            sumsq = ctmp.tile([1, CW], FP, tag="sumsq")
            sqt = ctmp.tile([128, FT, CW], FP, tag="sqt")
            for ft in range(FT):
                nc.vector.tensor_mul(out=sqt[:, ft, 1:1 + W], in0=cr[:, ft, 1:1 + W],
                                     in1=cr[:, ft, 1:1 + W])
            for (c0, cl) in chunks:
                pss = s_ps.tile([1, 512], FP, tag="pss")
                psq = s_ps.tile([1, 512], FP, tag="psq")
                for ft in range(FT):
                    nc.tensor.matmul(pss[:, :cl], ones_t[:], cr[:, ft, c0:c0 + cl],
                                     start=(ft == 0), stop=(ft == FT - 1))
                    nc.tensor.matmul(psq[:, :cl], ones_t[:], sqt[:, ft, c0:c0 + cl],
                                     start=(ft == 0), stop=(ft == FT - 1))
                nc.vector.tensor_copy(out=sums[:, c0:c0 + cl], in_=pss[:, :cl])
                nc.vector.tensor_copy(out=sumsq[:, c0:c0 + cl], in_=psq[:, :cl])
            mean = ctmp.tile([1, CW], FP, tag="mean")
            nc.vector.tensor_scalar(out=mean[:, 1:1 + W], in0=sums[:, 1:1 + W],
                                    scalar1=1.0 / H, scalar2=None, op0=ALU.mult)
            var = ctmp.tile([1, CW], FP, tag="var")
            nc.vector.tensor_mul(out=var[:, 1:1 + W], in0=mean[:, 1:1 + W],
                                 in1=mean[:, 1:1 + W])
            nc.vector.tensor_scalar(out=sumsq[:, 1:1 + W], in0=sumsq[:, 1:1 + W],
                                    scalar1=1.0 / H, scalar2=None, op0=ALU.mult)
            nc.vector.tensor_tensor(out=var[:, 1:1 + W], in0=sumsq[:, 1:1 + W],
                                    in1=var[:, 1:1 + W], op=ALU.subtract)
            rstd = ctmp.tile([1, CW], FP, tag="rstd")
            nc.scalar.activation(out=rstd[:, 1:1 + W], in_=var[:, 1:1 + W], func=AF.Sqrt,
                                 bias=eps_t[:1, :], scale=1.0)
            nc.vector.reciprocal(out=rstd[:, 1:1 + W], in_=rstd[:, 1:1 + W])
            # negms = -(mean * rstd); then c_norm = c * bcast(rstd) + bcast(negms)
            negms = ctmp.tile([1, CW], FP, tag="negms")
            nc.vector.tensor_mul(out=negms[:, 1:1 + W], in0=mean[:, 1:1 + W],
                                 in1=rstd[:, 1:1 + W])
            nc.vector.tensor_scalar(out=negms[:, 1:1 + W], in0=negms[:, 1:1 + W],
                                    scalar1=-1.0, scalar2=None, op0=ALU.mult)
            for (c0, cl) in chunks:
                rb = s_ps.tile([128, 512], FP, tag="rb")
                nc.tensor.matmul(rb[:, :cl], ones_row[:], rstd[:, c0:c0 + cl],
                                 start=True, stop=True)
                mb = s_ps.tile([128, 512], FP, tag="mb")
                nc.tensor.matmul(mb[:, :cl], ones_row[:], negms[:, c0:c0 + cl],
                                 start=True, stop=True)
                for ft in range(FT):
                    nc.vector.tensor_tensor(out=cr[:, ft, c0:c0 + cl],
                                            in0=cr[:, ft, c0:c0 + cl],
                                            in1=rb[:, :cl], op=ALU.mult)
                    nc.vector.tensor_tensor(out=cr[:, ft, c0:c0 + cl],
                                            in0=cr[:, ft, c0:c0 + cl],
                                            in1=mb[:, :cl], op=ALU.add)
                    if not flags["cn1"]:
                        nc.vector.tensor_scalar(out=cr[:, ft, c0:c0 + cl],
                                                in0=cr[:, ft, c0:c0 + cl],
                                                scalar1=cng_sb[:, j, ft, :],
                                                scalar2=cnb_sb[:, j, ft, :],
                                                op0=ALU.mult, op1=ALU.add)
            # re-zero the out-of-graph halo (cores 0/7): the reference
            # zero-pads at every conv layer, and conv smears real values
            # into the halo otherwise
            for ft in range(FT):
                nc.vector.tensor_scalar_mul(out=cr[:, ft, 0:HALO],
                                            in0=cr[:, ft, 0:HALO], scalar1=ml_sb[:])
                nc.vector.tensor_scalar_mul(out=cr[:, ft, CW - HALO:CW],
                                            in0=cr[:, ft, CW - HALO:CW], scalar1=mr_sb[:])
            c_fm = cr
        cps.close()
        if dbg:
            for ft in range(FT):
                nc.sync.dma_start(out=dbg_cnn[:, ft * CW:(ft + 1) * CW],
                                  in_=c_fm[:, ft, :])

        # ---- phase 3: fused projection + attention ----
        attn = ExitStack()
        aw = attn.enter_context(tc.tile_pool(name="attnw", bufs=1))
        q_sb = aw.tile([128, HEADS, NPC], FP)
        o_fm = aw.tile([128, HEADS, NPC], FP)

        qkvtmp = ExitStack()
        qtp = qkvtmp.enter_context(tc.tile_pool(name="qkvtmp", bufs=1))
        ipw_sb = qtp.tile([128, 4, 3 * E2], FP)
        for kt in range(4):
            nc.sync.dma_start(out=ipw_sb[:, kt, :], in_=ipwT[kt])
        kvt_sb = qtp.tile([128, 8, NPC], FP)
        if not flags["ipb0"]:
            ipb_sb = qtp.tile([128, 12, 1], FP)
            nc.sync.dma_start(out=ipb_sb[:], in_=ipb_in[:])
        v_nm = qtp.tile([128, NTC, E2], FP)

        qkv_ps = ExitStack()
        q_ps = qkv_ps.enter_context(tc.tile_pool(name="q_ps", bufs=3, space="PSUM"))
        qt_ps = qkv_ps.enter_context(tc.tile_pool(name="qt_ps", bufs=2, space="PSUM"))
        # fused_fm tiles: [gnn ho_fm (2) | cnn c_fm own (2)]
        fused = [ho_fm[:, 0, :], ho_fm[:, 1, :],
                 c_fm[:, 0, HALO:HALO + NPC], c_fm[:, 1, HALO:HALO + NPC]]
        for ot in range(12):
            psq = q_ps.tile([128, NPC], FP, tag="psq")
            for kt in range(4):
                nc.tensor.matmul(psq[:], ipw_sb[:, kt, ot * 128:(ot + 1) * 128],
                                 fused[kt], start=(kt == 0), stop=(kt == 3))
            dst = q_sb[:, ot, :] if ot < 4 else kvt_sb[:, ot - 4, :]
            if flags["ipb0"]:
                nc.vector.tensor_copy(out=dst, in_=psq[:])
            else:
                nc.vector.tensor_scalar(out=dst, in0=psq[:],
                                        scalar1=ipb_sb[:, ot, :], scalar2=None,
                                        op0=ALU.add)
        # v (kvt tiles 4..7) feature-major -> node-major for AV lhsT
        for nt in range(NTC):
            for vt in range(4):
                pt = qt_ps.tile([128, 128], FP, tag="qtps")
                nc.tensor.transpose(pt[:], kvt_sb[:, 4 + vt, nt * 128:(nt + 1) * 128],
                                    ident[:])
                nc.vector.tensor_copy(out=v_nm[:, nt, vt * 128:(vt + 1) * 128], in_=pt[:])
        qkv_ps.close()
        if dbg:
            for ot in range(12):
                srcq = q_sb[:, ot, :] if ot < 4 else kvt_sb[:, ot - 4, :]
                nc.sync.dma_start(out=dbg_qkv[:, ot * NPC:(ot + 1) * NPC], in_=srcq)

        kv_in = dram.tile([2 * E2, NPC], FP, name="kv_in")
        for kt in range(4):
            nc.sync.dma_start(out=kv_in[kt * 128:(kt + 1) * 128, :], in_=kvt_sb[:, kt, :])
        for nt in range(NTC):
            nc.sync.dma_start(out=kv_in[E2 + nt * 128:E2 + (nt + 1) * 128, :],
                              in_=v_nm[:, nt, :])
        kv_out = dram.tile([NCORES * 2 * E2, NPC], FP, name="kv_out", addr_space="Shared")
        nc.gpsimd.collective_compute("AllGather", ALU.bypass, replica_groups=RG,
                                     ins=[kv_in.opt()], outs=[kv_out.opt()])
        qkvtmp.close()

        heads_es = ExitStack()
        kv_pool = heads_es.enter_context(tc.tile_pool(name="kv", bufs=2))
        pt_pool = heads_es.enter_context(tc.tile_pool(name="ptp", bufs=2))
        at_ps = ExitStack()
        st_ps = at_ps.enter_context(tc.tile_pool(name="st_ps", bufs=2, space="PSUM"))
        o_ps = at_ps.enter_context(tc.tile_pool(name="o_ps", bufs=1, space="PSUM"))
        rs_ps = at_ps.enter_context(tc.tile_pool(name="rs_ps", bufs=1, space="PSUM"))
        inv_sqrt_hd = 1.0 / float(np.sqrt(HD))
        GRP = 3
        groups = [list(range(g, min(g + GRP, NT))) for g in range(0, NT, GRP)]
        for h in range(HEADS):
            k_sb = kv_pool.tile([128, N], FP, tag="k_sb")
            for r in range(NCORES):
                nc.sync.dma_start(out=k_sb[:, r * NPC:(r + 1) * NPC],
                                  in_=kv_out[r * 2 * E2 + h * 128:r * 2 * E2 + (h + 1) * 128, :])
            v_sb = kv_pool.tile([128, NT, 128], FP, tag="v_sb")
            for st in range(NT):
                r, t = st // NTC, st % NTC
                base = r * 2 * E2 + E2 + t * 128
                nc.sync.dma_start(out=v_sb[:, st, :],
                                  in_=kv_out[base:base + 128, h * 128:(h + 1) * 128])
            pso = o_ps.tile([128, NPC], FP, tag="pso")
            psr = rs_ps.tile([1, NPC], FP, tag="psr")
            for grp in groups:
                pst = st_ps.tile([128, GRP * NPC], FP, tag="pst")
                for gi, s in enumerate(grp):
                    nc.tensor.matmul(pst[:, gi * NPC:(gi + 1) * NPC],
                                     k_sb[:, s * 128:(s + 1) * 128],
                                     q_sb[:, h, :], start=True, stop=True)
                p_sb = pt_pool.tile([128, GRP * NPC], FP, tag="p_sb")
                nc.scalar.activation(out=p_sb[:, :len(grp) * NPC],
                                     in_=pst[:, :len(grp) * NPC],
                                     func=AF.Exp, scale=inv_sqrt_hd)
                for gi, s in enumerate(grp):
                    rhs = p_sb[:, gi * NPC:(gi + 1) * NPC]
                    nc.tensor.matmul(pso[:], v_sb[:, s, :], rhs,
                                     start=(s == 0), stop=(s == NT - 1))
                    nc.tensor.matmul(psr[:], ones_t[:], rhs,
                                     start=(s == 0), stop=(s == NT - 1))
            rr = pt_pool.tile([1, NPC], FP, tag="rr")
            nc.vector.tensor_copy(out=rr[:], in_=psr[:])
            nc.vector.reciprocal(out=rr[:], in_=rr[:])
            rb2 = rs_ps.tile([128, NPC], FP, tag="psr")
            nc.tensor.matmul(rb2[:], ones_row[:], rr[:], start=True, stop=True)
            ou_sb = pt_pool.tile([128, NPC], FP, tag="ou_sb")
            nc.vector.tensor_copy(out=ou_sb[:], in_=pso[:])
            nc.vector.tensor_tensor(out=o_fm[:, h, :], in0=ou_sb[:], in1=rb2[:],
                                    op=ALU.mult)
        at_ps.close()
        heads_es.close()
        if dbg:
            for h in range(HEADS):
                nc.sync.dma_start(out=dbg_ofm[:, h * NPC:(h + 1) * NPC],
                                  in_=o_fm[:, h, :])

        # ---- phase 4: out_proj + layernorm + fuse head ----
        tailw = ExitStack()
        tw = tailw.enter_context(tc.tile_pool(name="tailw", bufs=1))
        ttmp = tailw.enter_context(tc.tile_pool(name="ttmp", bufs=3))
        tl_ps = ExitStack()
        p_ps = tl_ps.enter_context(tc.tile_pool(name="p_ps", bufs=2, space="PSUM"))
        a_ps = tl_ps.enter_context(tc.tile_pool(name="a_ps", bufs=1, space="PSUM"))
        opw_sb = tw.tile([128, 4, E2], FP)
        for kt in range(4):
            nc.sync.dma_start(out=opw_sb[:, kt, :], in_=opwT[kt])
        fw_sb = tw.tile([128, 4, OUT], FP)
        for kt in range(4):
            nc.sync.dma_start(out=fw_sb[:, kt, :], in_=fwT[kt])
        if not flags["opb0"]:
            opb_sb = tw.tile([128, 4, 1], FP)
            nc.sync.dma_start(out=opb_sb[:], in_=opb_in[:])
        if not flags["an1"]:
            ang_sb = tw.tile([128, 4, 1], FP)
            anb_sb = tw.tile([128, 4, 1], FP)
            nc.sync.dma_start(out=ang_sb[:], in_=ang_in[:])
            nc.sync.dma_start(out=anb_sb[:], in_=anb_in[:])
        if not flags["fb0"]:
            fb_sb = tw.tile([64, 1], FP)
            nc.sync.dma_start(out=fb_sb[:], in_=fb_in[:])

        op_sb = tw.tile([128, 4, NPC], FP)
        for ot in range(4):
            psp = p_ps.tile([128, NPC], FP, tag="psp")
            for kt in range(4):
                nc.tensor.matmul(psp[:], opw_sb[:, kt, ot * 128:(ot + 1) * 128],
                                 o_fm[:, kt, :], start=(kt == 0), stop=(kt == 3))
            if flags["opb0"]:
                nc.vector.tensor_copy(out=op_sb[:, ot, :], in_=psp[:])
            else:
                nc.vector.tensor_scalar(out=op_sb[:, ot, :], in0=psp[:],
                                        scalar1=opb_sb[:, ot, :], scalar2=None, op0=ALU.add)
        # anorm LN over 512 features (partition reduce over 4 tiles)
        pss = a_ps.tile([1, NPC], FP, tag="pssa")
        psq2 = a_ps.tile([1, NPC], FP, tag="psqa")
        sq2 = ttmp.tile([128, 4, NPC], FP, tag="sq2")
        for kt in range(4):
            nc.vector.tensor_mul(out=sq2[:, kt, :], in0=op_sb[:, kt, :], in1=op_sb[:, kt, :])
        for kt in range(4):
            nc.tensor.matmul(pss[:], ones_t[:], op_sb[:, kt, :],
                             start=(kt == 0), stop=(kt == 3))
            nc.tensor.matmul(psq2[:], ones_t[:], sq2[:, kt, :],
                             start=(kt == 0), stop=(kt == 3))
        mean2 = ttmp.tile([1, NPC], FP, tag="mean2")
        nc.vector.tensor_scalar(out=mean2[:], in0=pss[:], scalar1=1.0 / E2, scalar2=None,
                                op0=ALU.mult)
        var2 = ttmp.tile([1, NPC], FP, tag="var2")
        nc.vector.tensor_mul(out=var2[:], in0=mean2[:], in1=mean2[:])
        sq_m = ttmp.tile([1, NPC], FP, tag="sq_m")
        nc.vector.tensor_scalar(out=sq_m[:], in0=psq2[:], scalar1=1.0 / E2, scalar2=None,
                                op0=ALU.mult)
        nc.vector.tensor_tensor(out=var2[:], in0=sq_m[:], in1=var2[:], op=ALU.subtract)
        # rstd = exp(-0.5 * ln(var + eps)) (keeps us in the ln/exp ACT table set)
        rstd2 = ttmp.tile([1, NPC], FP, tag="rstd2")
        nc.scalar.activation(out=rstd2[:], in_=var2[:], func=AF.Ln, bias=eps_t[:1, :],
                             scale=1.0)
        nc.scalar.activation(out=rstd2[:], in_=rstd2[:], func=AF.Exp, scale=-0.5)
        negm2 = ttmp.tile([1, NPC], FP, tag="negm2")
        nc.vector.tensor_mul(out=negm2[:], in0=mean2[:], in1=rstd2[:])
        nc.vector.tensor_scalar(out=negm2[:], in0=negm2[:], scalar1=-1.0, scalar2=None,
                                op0=ALU.mult)
        rb3 = a_ps.tile([128, NPC], FP, tag="pssa")
        nc.tensor.matmul(rb3[:], ones_row[:], rstd2[:], start=True, stop=True)
        mb3 = a_ps.tile([128, NPC], FP, tag="psqa")
        nc.tensor.matmul(mb3[:], ones_row[:], negm2[:], start=True, stop=True)
        for kt in range(4):
            nc.vector.tensor_tensor(out=op_sb[:, kt, :], in0=op_sb[:, kt, :],
                                    in1=rb3[:], op=ALU.mult)
            nc.vector.tensor_tensor(out=op_sb[:, kt, :], in0=op_sb[:, kt, :],
                                    in1=mb3[:], op=ALU.add)
            if not flags["an1"]:
                nc.vector.tensor_scalar(out=op_sb[:, kt, :], in0=op_sb[:, kt, :],
                                        scalar1=ang_sb[:, kt, :], scalar2=anb_sb[:, kt, :],
                                        op0=ALU.mult, op1=ALU.add)
        if dbg:
            for kt in range(4):
                nc.sync.dma_start(out=dbg_opn[:, kt * NPC:(kt + 1) * NPC],
                                  in_=op_sb[:, kt, :])
        # fuse head: [64, 512] then transpose to node-major output
        psf = p_ps.tile([64, NPC], FP, tag="psf")
        for kt in range(4):
            nc.tensor.matmul(psf[:], fw_sb[:, kt, :], op_sb[:, kt, :],
                             start=(kt == 0), stop=(kt == 3))
        ff_sb = ttmp.tile([64, NPC], FP, tag="ff_sb")
        if flags["fb0"]:
            nc.vector.tensor_copy(out=ff_sb[:], in_=psf[:])
        else:
            nc.vector.tensor_scalar(out=ff_sb[:], in0=psf[:], scalar1=fb_sb[:],
                                    scalar2=None, op0=ALU.add)
        for nt in range(NTC):
            ptf = a_ps.tile([128, 64], FP, tag="ptf")
            nc.tensor.matmul(ptf[:], ff_sb[:, nt * 128:(nt + 1) * 128], ident[:64, :64],
                             is_transpose=True)
            of_sb = ttmp.tile([128, 64], FP, tag="of_sb")
            nc.vector.tensor_copy(out=of_sb[:], in_=ptf[:])
            nc.sync.dma_start(out=out_d[nt * 128:(nt + 1) * 128, :], in_=of_sb[:])
        tailw.close()
        tl_ps.close()
        attn.close()
        cnn.close()
        stack.close()

    nc.compile()
    return nc


def cr_prev_slice(c_fm, it, c0, cl):
    return c_fm[:, it, c0:c0 + cl]


def _prep(inputs):
    """Host-side shard/transform. Returns per-core in_maps + flags."""
    x = np.ascontiguousarray(np.asarray(inputs["x"], np.float32))
    ei = np.asarray(inputs["edge_index"], np.int64)
    src, dst = ei[0], ei[1]
    deg = np.bincount(dst, minlength=N).astype(np.float32)
    deg = np.maximum(deg, 1.0)
    A = np.zeros((N, N), np.float32)
    np.add.at(A, (dst, src), 1.0)
    A /= deg[:, None]

    sage_wl = np.asarray(inputs["sage_wl"], np.float32)
    sage_wr = np.asarray(inputs["sage_wr"], np.float32)
    wlT = np.ascontiguousarray(
        sage_wl.transpose(0, 2, 1).reshape(L, FT, 128, H))
    wrT = np.ascontiguousarray(
        sage_wr.transpose(0, 2, 1).reshape(L, FT, 128, H))
    conv_w = np.asarray(inputs["conv_w"], np.float32)       # [3, O, I, K]
    cwT = np.ascontiguousarray(
        conv_w.transpose(0, 3, 2, 1).reshape(3, 3, FT, 128, H))
    ipwT = np.ascontiguousarray(
        np.asarray(inputs["in_proj_w"], np.float32).T.reshape(4, 128, 3 * E2))
    opwT = np.ascontiguousarray(
        np.asarray(inputs["out_proj_w"], np.float32).T.reshape(4, 128, E2))
    fwT = np.ascontiguousarray(
        np.asarray(inputs["fuse_w"], np.float32).T.reshape(4, 128, OUT))

    bl = np.asarray(inputs["sage_bl"], np.float32)
    lng = np.asarray(inputs["ln_g"], np.float32)
    lnb = np.asarray(inputs["ln_b"], np.float32)
    cb = np.asarray(inputs["conv_b"], np.float32)
    cng = np.asarray(inputs["cnorm_g"], np.float32)
    cnb = np.asarray(inputs["cnorm_b"], np.float32)
    ipb = np.asarray(inputs["in_proj_b"], np.float32)
    opb = np.asarray(inputs["out_proj_b"], np.float32)
    ang = np.asarray(inputs["anorm_g"], np.float32)
    anb = np.asarray(inputs["anorm_b"], np.float32)
    fb = np.asarray(inputs["fuse_b"], np.float32)
    flags = {
        "bl0": bool(np.all(bl == 0)),
        "ln1": bool(np.all(lng == 1) and np.all(lnb == 0)),
        "cb0": bool(np.all(cb == 0)),
        "cn1": bool(np.all(cng == 1) and np.all(cnb == 0)),
        "ipb0": bool(np.all(ipb == 0)),
        "opb0": bool(np.all(opb == 0)),
        "an1": bool(np.all(ang == 1) and np.all(anb == 0)),
        "fb0": bool(np.all(fb == 0)),
    }

    in_maps = []
    for c in range(NCORES):
        rows = slice(c * NPC, (c + 1) * NPC)
        a_ct = np.ascontiguousarray(A[rows].T.reshape(NT, 128, NPC))
        lo = c * NPC - HALO
        idx_l = np.clip(np.arange(lo, lo + HALO), 0, N - 1).astype(np.int32)
        hi = (c + 1) * NPC
        idx_r = np.clip(np.arange(hi, hi + HALO), 0, N - 1).astype(np.int32)
        m = {
            "x_full": x,
            "x_own": np.ascontiguousarray(x[rows]),
            "a_ct": a_ct,
            "wlT": wlT, "wrT": wrT, "cwT": cwT,
            "ipwT": ipwT, "opwT": opwT, "fwT": fwT,
            "idx_l": idx_l.reshape(HALO, 1),
            "idx_r": idx_r.reshape(HALO, 1),
            "mask_l": np.full((HALO, 1), 0.0 if c == 0 else 1.0, np.float32),
            "mask_r": np.full((HALO, 1), 0.0 if c == NCORES - 1 else 1.0, np.float32),
        }
        if not flags["bl0"]:
            m["bl_in"] = np.ascontiguousarray(bl.reshape(1, L, H))
        if not flags["ln1"]:
            m["lng_in"] = np.ascontiguousarray(lng.reshape(1, L - 1, H))
            m["lnb_in"] = np.ascontiguousarray(lnb.reshape(1, L - 1, H))
        if not flags["cb0"]:
            m["cb_in"] = np.ascontiguousarray(
                cb.reshape(3, FT, 128).transpose(2, 0, 1).reshape(128, 3, FT, 1))
        if not flags["cn1"]:
            m["cng_in"] = np.ascontiguousarray(
                cng.reshape(3, FT, 128).transpose(2, 0, 1).reshape(128, 3, FT, 1))
            m["cnb_in"] = np.ascontiguousarray(
                cnb.reshape(3, FT, 128).transpose(2, 0, 1).reshape(128, 3, FT, 1))
        if not flags["ipb0"]:
            m["ipb_in"] = np.ascontiguousarray(
                ipb.reshape(12, 128).T.reshape(128, 12, 1))
        if not flags["opb0"]:
            m["opb_in"] = np.ascontiguousarray(
                opb.reshape(4, 128).T.reshape(128, 4, 1))
        if not flags["an1"]:
            m["ang_in"] = np.ascontiguousarray(
                ang.reshape(4, 128).T.reshape(128, 4, 1))
            m["anb_in"] = np.ascontiguousarray(
                anb.reshape(4, 128).T.reshape(128, 4, 1))
        if not flags["fb0"]:
            m["fb_in"] = np.ascontiguousarray(fb.reshape(OUT, 1))
        in_maps.append(m)
    return in_maps, flags


last_exec_time_ns = None


last_debug = None


def kernel(**inputs) -> np.ndarray:
    global last_exec_time_ns, last_debug
    import os
    dbg = os.environ.get("KERNEL_DEBUG", "0") == "1"
    in_maps, flags = _prep(inputs)
    key = (dbg,) + tuple(sorted(flags.items()))
    if key not in _CACHE:
        _CACHE[key] = _build(flags, dbg=dbg)
    nc = _CACHE[key]
    trace = os.environ.get("KERNEL_TRACE", "0") == "1"
    res = run_bass_kernel_spmd(nc, in_maps, core_ids=list(range(NCORES)),
                               trace=trace)
    last_exec_time_ns = res.exec_time_ns
    if dbg:
        last_debug = res.results
    out = np.concatenate([r["out"] for r in res.results], axis=0)
    return out.astype(np.float32)


# revision 16
# speedup vs baseline: 1.0681x; 1.0681x over previous
# Trainium2 Bass kernel for AdvancedHybridHOIGNN (6x SAGEConv + 3x Conv1d + MHA + fuse).
#
# Sharding: 4096 nodes split 512/core across 8 cores. The SAGE neighbor
# aggregation is done as dense matmuls against a host-built normalized
# adjacency slice (A[dst, src]/deg, transposed, per core). Node features are
# replicated each layer via AllGather. The CNN branch needs a 128-node halo,
# gathered with indirect DMA using per-core index data so the single SPMD
# program stays uniform across cores. Attention is sequence-parallel over
# queries with AllGathered K/V.
#
# Layouts: activations flow in both node-major ([node_part, feat]) and
# feature-major ([feat_part, node]) forms; feature-major feeds matmul lhsT
# slices, node-major feeds LayerNorm (bn_stats reduces along free axis).

import sys

sys.path.insert(0, "/opt/trn_rl_repo")

import numpy as np

import concourse.bass as bass
import concourse.mybir as mybir
import concourse.tile as tile
from concourse import bacc
from concourse.bass_utils import run_bass_kernel_spmd
from concourse.masks import make_identity

FP = mybir.dt.float32
AF = mybir.ActivationFunctionType
ALU = mybir.AluOpType

N, H, OUT, L = 4096, 256, 64, 6
NCORES = 8
NPC = N // NCORES          # 512 nodes per core
NT = N // 128              # 32 node tiles globally
NTC = NPC // 128           # 4 node tiles per core
FT = H // 128              # 2 feature tiles
E2 = 2 * H                 # 512
HEADS, HD = 4, 128
HALO = 128                 # halo width for the conv branch (needs only 3)
CW = NPC + 2 * HALO        # 768: conv working width per core
EPS = 1e-5

_CACHE = {}


def _build(flags, dbg=False):
    """Trace + compile the SPMD Bass program. flags: which affine params are
    trivial (zeros/ones) and can be skipped."""
    nc = bacc.Bacc("TRN2", target_bir_lowering=False, debug=False,
                   num_devices=NCORES)
    RG = [list(range(NCORES))]

    # ---------------- kernel I/O ----------------
    x_full = nc.dram_tensor("x_full", [N, H], FP, kind="ExternalInput")
    x_own = nc.dram_tensor("x_own", [NPC, H], FP, kind="ExternalInput")
    a_ct = nc.dram_tensor("a_ct", [NT, 128, NPC], FP, kind="ExternalInput")
    wlT = nc.dram_tensor("wlT", [L, FT, 128, H], FP, kind="ExternalInput")
    wrT = nc.dram_tensor("wrT", [L, FT, 128, H], FP, kind="ExternalInput")
    cwT = nc.dram_tensor("cwT", [3, 3, FT, 128, H], FP, kind="ExternalInput")
    ipwT = nc.dram_tensor("ipwT", [4, 128, 3 * E2], FP, kind="ExternalInput")
    opwT = nc.dram_tensor("opwT", [4, 128, E2], FP, kind="ExternalInput")
    fwT = nc.dram_tensor("fwT", [4, 128, OUT], FP, kind="ExternalInput")
    idx_l = nc.dram_tensor("idx_l", [HALO, 1], mybir.dt.int32, kind="ExternalInput")
    idx_r = nc.dram_tensor("idx_r", [HALO, 1], mybir.dt.int32, kind="ExternalInput")
    mask_l = nc.dram_tensor("mask_l", [HALO, 1], FP, kind="ExternalInput")
    mask_r = nc.dram_tensor("mask_r", [HALO, 1], FP, kind="ExternalInput")
    # optional affine params (only staged when nontrivial)
    if not flags["bl0"]:
        bl_in = nc.dram_tensor("bl_in", [1, L, H], FP, kind="ExternalInput")
    if not flags["ln1"]:
        lng_in = nc.dram_tensor("lng_in", [1, L - 1, H], FP, kind="ExternalInput")
        lnb_in = nc.dram_tensor("lnb_in", [1, L - 1, H], FP, kind="ExternalInput")
    if not flags["cb0"]:
        cb_in = nc.dram_tensor("cb_in", [128, 3, FT, 1], FP, kind="ExternalInput")
    if not flags["cn1"]:
        cng_in = nc.dram_tensor("cng_in", [128, 3, FT, 1], FP, kind="ExternalInput")
        cnb_in = nc.dram_tensor("cnb_in", [128, 3, FT, 1], FP, kind="ExternalInput")
    if not flags["ipb0"]:
        ipb_in = nc.dram_tensor("ipb_in", [128, 12, 1], FP, kind="ExternalInput")
    if not flags["opb0"]:
        opb_in = nc.dram_tensor("opb_in", [128, 4, 1], FP, kind="ExternalInput")
    if not flags["an1"]:
        ang_in = nc.dram_tensor("ang_in", [128, 4, 1], FP, kind="ExternalInput")
        anb_in = nc.dram_tensor("anb_in", [128, 4, 1], FP, kind="ExternalInput")
    if not flags["fb0"]:
        fb_in = nc.dram_tensor("fb_in", [64, 1], FP, kind="ExternalInput")
    out_d = nc.dram_tensor("out", [NPC, OUT], FP, kind="ExternalOutput")
    if dbg:
        dbg_gnn = nc.dram_tensor("dbg_gnn", [NPC, H], FP, kind="ExternalOutput")
        dbg_cnn = nc.dram_tensor("dbg_cnn", [128, FT * CW], FP, kind="ExternalOutput")
        dbg_qkv = nc.dram_tensor("dbg_qkv", [128, 12 * NPC], FP, kind="ExternalOutput")
        dbg_ofm = nc.dram_tensor("dbg_ofm", [128, HEADS * NPC], FP, kind="ExternalOutput")
        dbg_opn = nc.dram_tensor("dbg_opn", [128, 4 * NPC], FP, kind="ExternalOutput")

    from contextlib import ExitStack

    with tile.TileContext(nc) as tc:
        stack = ExitStack()
        # long-lived pools on the right SBUF side; phase pools stack LIFO on the left
        singles = stack.enter_context(tc.tile_pool(name="singles", bufs=1, side="right"))
        dram = stack.enter_context(tc.tile_pool(name="dram", bufs=1, space="DRAM"))

        ident = singles.tile([128, 128], FP)
        make_identity(nc, ident[:])
        eps_t = singles.tile([128, 1], FP)
        nc.vector.memset(eps_t[:], EPS)
        ones_t = singles.tile([128, 1], FP)
        nc.vector.memset(ones_t[:], 1.0)
        ones_row = singles.tile([1, 128], FP)
        nc.vector.memset(ones_row[:], 1.0)

        # ---- phase 1: SAGE layers ----
        sagew = ExitStack()
        sw = sagew.enter_context(tc.tile_pool(name="sagew", bufs=1))
        act_sb = sw.tile([128, NT, NPC], FP)          # A_cT resident (64KB/part)
        for s in range(NT):
            nc.sync.dma_start(out=act_sb[:, s, :], in_=a_ct[s])
        wl_sb = sw.tile([128, L, FT, H], FP)
        wr_sb = sw.tile([128, L, FT, H], FP)
        for i in range(L):
            for ft in range(FT):
                nc.sync.dma_start(out=wl_sb[:, i, ft, :], in_=wlT[i, ft])
                nc.sync.dma_start(out=wr_sb[:, i, ft, :], in_=wrT[i, ft])
        if not flags["bl0"]:
            bl_sb = sw.tile([128, L, H], FP)
            nc.gpsimd.dma_start(out=bl_sb[:], in_=bl_in[:].to_broadcast([128, L, H]))
        if not flags["ln1"]:
            lng_sb = sw.tile([128, L - 1, H], FP)
            lnb_sb = sw.tile([128, L - 1, H], FP)
            nc.gpsimd.dma_start(out=lng_sb[:], in_=lng_in[:].to_broadcast([128, L - 1, H]))
            nc.gpsimd.dma_start(out=lnb_sb[:], in_=lnb_in[:].to_broadcast([128, L - 1, H]))

        ho_pool = stack.enter_context(tc.tile_pool(name="ho", bufs=2, side="right"))
        hs_pool = ExitStack()
        hstream = hs_pool.enter_context(tc.tile_pool(name="hstream", bufs=12))
        sage_ps = ExitStack()
        agg_ps = sage_ps.enter_context(tc.tile_pool(name="agg_ps", bufs=1, space="PSUM"))
        z_ps = sage_ps.enter_context(tc.tile_pool(name="z_ps", bufs=2, space="PSUM"))
        t_ps = sage_ps.enter_context(tc.tile_pool(name="t_ps", bufs=2, space="PSUM"))
        sage_tmp = ExitStack()
        stmp = sage_tmp.enter_context(tc.tile_pool(name="stmp", bufs=4))
        aggp = sage_tmp.enter_context(tc.tile_pool(name="aggsb", bufs=2))

        # initial own-slice: node-major + feature-major
        ho_nm = ho_pool.tile([128, NTC, H], FP, name="ho_nm0", tag="ho_nm")
        for nt in range(NTC):
            nc.sync.dma_start(out=ho_nm[:, nt, :], in_=x_own[nt * 128:(nt + 1) * 128, :])
        ho_fm = ho_pool.tile([128, FT, NPC], FP, name="ho_fm0", tag="ho_fm")
        for nt in range(NTC):
            for ft in range(FT):
                pt = t_ps.tile([128, 128], FP, tag="tps")
                nc.tensor.transpose(pt[:], ho_nm[:, nt, ft * 128:(ft + 1) * 128], ident[:])
                nc.vector.tensor_copy(out=ho_fm[:, ft, nt * 128:(nt + 1) * 128], in_=pt[:])

        cc_outs = []   # per layer: list of FT chunk buffers [N, 128]
        cc_halo_out = None
        for i in range(L):
            # aggregation: agg_fm[f, d] = sum_s h[s, f] * A_cT[s, d]
            # h arrives in FT feature chunks so the AllGather of chunk ft+1
            # overlaps the matmuls of chunk ft
            psa = [agg_ps.tile([128, NPC], FP, name=f"psa{i}_{ft}", tag=f"psa{ft}")
                   for ft in range(FT)]
            for ft in range(FT):
                for s in range(NT):
                    hk = hstream.tile([128, 128], FP, tag="hk")
                    if i == 0:
                        nc.sync.dma_start(
                            out=hk[:],
                            in_=x_full[s * 128:(s + 1) * 128, ft * 128:(ft + 1) * 128])
                    else:
                        nc.sync.dma_start(
                            out=hk[:], in_=cc_outs[i - 1][ft][s * 128:(s + 1) * 128, :])
                    nc.tensor.matmul(psa[ft][:], hk[:], act_sb[:, s, :],
                                     start=(s == 0), stop=(s == NT - 1))
            agg_fm = aggp.tile([128, FT, NPC], FP, tag="agg_fm")
            for ft in range(FT):
                nc.vector.tensor_copy(out=agg_fm[:, ft, :], in_=psa[ft][:])

            # z[n, o] = agg @ wl.T + h @ wr.T  (node-major out)
            ho_nm_new = ho_pool.tile([128, NTC, H], FP, name=f"ho_nm{i + 1}", tag="ho_nm")
            ho_fm_new = ho_pool.tile([128, FT, NPC], FP, name=f"ho_fm{i + 1}", tag="ho_fm")
            for nt in range(NTC):
                ns = slice(nt * 128, (nt + 1) * 128)
                psz = z_ps.tile([128, H], FP, tag="psz")
                for ft in range(FT):
                    nc.tensor.matmul(psz[:], agg_fm[:, ft, ns], wl_sb[:, i, ft, :],
                                     start=(ft == 0), stop=False)
                for ft in range(FT):
                    nc.tensor.matmul(psz[:], ho_fm[:, ft, ns], wr_sb[:, i, ft, :],
                                     start=False, stop=(ft == FT - 1))
                z_sb = stmp.tile([128, H], FP, tag="z_sb")
                if flags["bl0"]:
                    nc.vector.tensor_copy(out=z_sb[:], in_=psz[:])
                else:
                    nc.vector.tensor_tensor(out=z_sb[:], in0=psz[:],
                                            in1=bl_sb[:, i, :], op=ALU.add)
                if i < L - 1:
                    stat = stmp.tile([128, 6], FP, tag="stat")
                    nc.vector.bn_stats(out=stat[:], in_=z_sb[:])
                    mv = stmp.tile([128, 2], FP, tag="mv")
                    nc.vector.bn_aggr(out=mv[:], in_=stat[:])
                    sd = stmp.tile([128, 1], FP, tag="sd")
                    nc.scalar.activation(out=sd[:], in_=mv[:, 1:2], func=AF.Sqrt,
                                         bias=eps_t[:], scale=1.0)
                    nc.vector.reciprocal(out=sd[:], in_=sd[:])
                    zn = stmp.tile([128, H], FP, tag="zn")
                    nc.vector.tensor_scalar(out=zn[:], in0=z_sb[:], scalar1=mv[:, 0:1],
                                            scalar2=sd[:], op0=ALU.subtract, op1=ALU.mult)
                    if not flags["ln1"]:
                        nc.vector.tensor_tensor(out=zn[:], in0=zn[:],
                                                in1=lng_sb[:, i, :], op=ALU.mult)
                        nc.vector.tensor_tensor(out=zn[:], in0=zn[:],
                                                in1=lnb_sb[:, i, :], op=ALU.add)
                else:
                    zn = z_sb
                zr = stmp.tile([128, H], FP, tag="zr")
                nc.scalar.activation(out=zr[:], in_=zn[:], func=AF.Relu)
                nc.vector.tensor_add(out=ho_nm_new[:, nt, :], in0=zr[:], in1=ho_nm[:, nt, :])
                for ft in range(FT):
                    pt = t_ps.tile([128, 128], FP, tag="tps")
                    nc.tensor.transpose(pt[:], ho_nm_new[:, nt, ft * 128:(ft + 1) * 128], ident[:])
                    nc.vector.tensor_copy(out=ho_fm_new[:, ft, ns], in_=pt[:])
            ho_nm, ho_fm = ho_nm_new, ho_fm_new

            if i < L - 1:
                # AllGather the updated slice, in FT feature chunks
                chunk_outs = []
                for ft in range(FT):
                    cc_in = dram.tile([NPC, 128], FP, name=f"cc_in{i}_{ft}")
                    for nt in range(NTC):
                        nc.sync.dma_start(out=cc_in[nt * 128:(nt + 1) * 128, :],
                                          in_=ho_nm[:, nt, ft * 128:(ft + 1) * 128])
                    cc_out = dram.tile([N, 128], FP, name=f"cc_out{i}_{ft}",
                                       addr_space="Shared")
                    nc.gpsimd.collective_compute(
                        "AllGather", ALU.bypass, replica_groups=RG,
                        ins=[cc_in.opt()], outs=[cc_out.opt()])
                    chunk_outs.append(cc_out)
                cc_outs.append(chunk_outs)
            else:
                # last layer: only the conv halo needs neighbours - exchange
                # just the first/last 128-node tiles
                cc_halo_in = dram.tile([2 * 128, H], FP, name="cc_halo_in")
                nc.sync.dma_start(out=cc_halo_in[0:128, :], in_=ho_nm[:, 0, :])
                nc.sync.dma_start(out=cc_halo_in[128:256, :], in_=ho_nm[:, NTC - 1, :])
                cc_halo_out = dram.tile([NCORES * 2 * 128, H], FP, name="cc_halo_out",
                                        addr_space="Shared")
                nc.gpsimd.collective_compute(
                    "AllGather", ALU.bypass, replica_groups=RG,
                    ins=[cc_halo_in.opt()], outs=[cc_halo_out.opt()])

        sage_tmp.close()
        hs_pool.close()
        sagew.close()
        sage_ps.close()
        if dbg:
            for nt in range(NTC):
                nc.sync.dma_start(out=dbg_gnn[nt * 128:(nt + 1) * 128, :],
                                  in_=ho_nm[:, nt, :])

        # ---- phase 2: CNN branch (feature-major, nodes on free axis) ----
        cnn = ExitStack()
        cw_pool = cnn.enter_context(tc.tile_pool(name="cnnw", bufs=1))
        cfm_pool = cnn.enter_context(tc.tile_pool(name="cfm", bufs=2))
        ctmp = cnn.enter_context(tc.tile_pool(name="ctmp", bufs=1))
        cps = ExitStack()
        c_ps = cps.enter_context(tc.tile_pool(name="c_ps", bufs=2, space="PSUM"))
        s_ps = cps.enter_context(tc.tile_pool(name="s_ps", bufs=1, space="PSUM"))
        ctp_es = ExitStack()
        ct_ps = ctp_es.enter_context(tc.tile_pool(name="ct_ps", bufs=2, space="PSUM"))

        cw_sb = cw_pool.tile([128, 3, 3, FT, H], FP)
        for j in range(3):
            for k in range(3):
                for ft in range(FT):
                    nc.sync.dma_start(out=cw_sb[:, j, k, ft, :], in_=cwT[j, k, ft])
        if not flags["cb0"]:
            cb_sb = cw_pool.tile([128, 3, FT, 1], FP)
            nc.sync.dma_start(out=cb_sb[:], in_=cb_in[:])
        if not flags["cn1"]:
            cng_sb = cw_pool.tile([128, 3, FT, 1], FP)
            cnb_sb = cw_pool.tile([128, 3, FT, 1], FP)
            nc.sync.dma_start(out=cng_sb[:], in_=cng_in[:])
            nc.sync.dma_start(out=cnb_sb[:], in_=cnb_in[:])

        # window: [left halo | own 512 | right halo] node-major then transpose
        il_sb = cw_pool.tile([HALO, 1], mybir.dt.int32)
        ir_sb = cw_pool.tile([HALO, 1], mybir.dt.int32)
        ml_sb = cw_pool.tile([HALO, 1], FP)
        mr_sb = cw_pool.tile([HALO, 1], FP)
        nc.sync.dma_start(out=il_sb[:], in_=idx_l[:])
        nc.sync.dma_start(out=ir_sb[:], in_=idx_r[:])
        nc.sync.dma_start(out=ml_sb[:], in_=mask_l[:])
        nc.sync.dma_start(out=mr_sb[:], in_=mask_r[:])
        halo_l = ctmp.tile([HALO, H], FP, tag="halo")
        nc.gpsimd.indirect_dma_start(
            out=halo_l[:], out_offset=None, in_=cc_halo_out[:],
            in_offset=bass.IndirectOffsetOnAxis(ap=il_sb[:, :1], axis=0))
        nc.vector.tensor_scalar_mul(out=halo_l[:], in0=halo_l[:], scalar1=ml_sb[:])
        halo_r = ctmp.tile([HALO, H], FP, tag="halo")
        nc.gpsimd.indirect_dma_start(
            out=halo_r[:], out_offset=None, in_=cc_halo_out[:],
            in_offset=bass.IndirectOffsetOnAxis(ap=ir_sb[:, :1], axis=0))
        nc.vector.tensor_scalar_mul(out=halo_r[:], in0=halo_r[:], scalar1=mr_sb[:])

        c_fm = cfm_pool.tile([128, FT, CW], FP, tag="c_fm", name="c_fm_in")
        wnd = [halo_l[:]] + [ho_nm[:, nt, :] for nt in range(NTC)] + [halo_r[:]]
        for w, src in enumerate(wnd):
            for ft in range(FT):
                pt = ct_ps.tile([128, 128], FP, tag="ctps")
                nc.tensor.transpose(pt[:], src[:, ft * 128:(ft + 1) * 128], ident[:])
                nc.vector.tensor_copy(out=c_fm[:, ft, w * 128:(w + 1) * 128], in_=pt[:])
        ctp_es.close()

        # conv layers: compute output cols [1, CW-1)
        chunks = [(1, 512), (513, CW - 1 - 513)]
        for j in range(3):
            cr = cfm_pool.tile([128, FT, CW], FP, tag="c_fm", name=f"c_fm{j}")
            for ft in range(FT):  # guard stale edge cols
                nc.vector.memset(cr[:, ft, 0:1], 0.0)
                nc.vector.memset(cr[:, ft, CW - 1:CW], 0.0)
            for ot in range(FT):
                for (c0, cl) in chunks:
                    psc = c_ps.tile([128, 512], FP, tag="psc")
                    first = True
                    for k in range(3):
                        for it in range(FT):
                            nc.tensor.matmul(
                                psc[:, :cl],
                                cw_sb[:, j, k, it, ot * 128:(ot + 1) * 128],
                                cr_prev_slice(c_fm, it, c0 + k - 1, cl),
                                start=first, stop=(k == 2 and it == FT - 1))
                            first = False
                    if flags["cb0"]:
                        nc.scalar.activation(out=cr[:, ot, c0:c0 + cl], in_=psc[:, :cl],
                                             func=AF.Relu)
                    else:
                        nc.scalar.activation(out=cr[:, ot, c0:c0 + cl], in_=psc[:, :cl],
                                             func=AF.Relu, bias=cb_sb[:, j, ot, :], scale=1.0)
            # channel LayerNorm per node (partition reduce via ones-matmul)
            W = CW - 2
            sums = ctmp.tile([1, CW], FP, tag="sums")
            sumsq = ctmp.tile([1, CW], FP, tag="sumsq")
            sqt = ctmp.tile([128, FT, CW], FP, tag="sqt")
            for ft in range(FT):
                nc.vector.tensor_mul(out=sqt[:, ft, 1:1 + W], in0=cr[:, ft, 1:1 + W],
                                     in1=cr[:, ft, 1:1 + W])
            for (c0, cl) in chunks:
                pss = s_ps.tile([1, 512], FP, tag="pss")
                psq = s_ps.tile([1, 512], FP, tag="psq")
                for ft in range(FT):
                    nc.tensor.matmul(pss[:, :cl], ones_t[:], cr[:, ft, c0:c0 + cl],
                                     start=(ft == 0), stop=(ft == FT - 1))
                    nc.tensor.matmul(psq[:, :cl], ones_t[:], sqt[:, ft, c0:c0 + cl],
                                     start=(ft == 0), stop=(ft == FT - 1))
                nc.vector.tensor_copy(out=sums[:, c0:c0 + cl], in_=pss[:, :cl])
                nc.vector.tensor_copy(out=sumsq[:, c0:c0 + cl], in_=psq[:, :cl])
            mean = ctmp.tile([1, CW], FP, tag="mean")
            nc.vector.tensor_scalar(out=mean[:, 1:1 + W], in0=sums[:, 1:1 + W],
                                    scalar1=1.0 / H, scalar2=None, op0=ALU.mult)
            var = ctmp.tile([1, CW], FP, tag="var")
            nc.vector.tensor_mul(out=var[:, 1:1 + W], in0=mean[:, 1:1 + W],
                                 in1=mean[:, 1:1 + W])
            nc.vector.tensor_scalar(out=sumsq[:, 1:1 + W], in0=sumsq[:, 1:1 + W],
                                    scalar1=1.0 / H, scalar2=None, op0=ALU.mult)
            nc.vector.tensor_tensor(out=var[:, 1:1 + W], in0=sumsq[:, 1:1 + W],
                                    in1=var[:, 1:1 + W], op=ALU.subtract)
            rstd = ctmp.tile([1, CW], FP, tag="rstd")
            nc.scalar.activation(out=rstd[:, 1:1 + W], in_=var[:, 1:1 + W], func=AF.Sqrt,
                                 bias=eps_t[:1, :], scale=1.0)
            nc.vector.reciprocal(out=rstd[:, 1:1 + W], in_=rstd[:, 1:1 + W])
            # negms = -(mean * rstd); then c_norm = c * bcast(rstd) + bcast(negms)
            negms = ctmp.tile([1, CW], FP, tag="negms")
            nc.vector.tensor_mul(out=negms[:, 1:1 + W], in0=mean[:, 1:1 + W],
                                 in1=rstd[:, 1:1 + W])
            nc.vector.tensor_scalar(out=negms[:, 1:1 + W], in0=negms[:, 1:1 + W],
                                    scalar1=-1.0, scalar2=None, op0=ALU.mult)
            for (c0, cl) in chunks:
                rb = s_ps.tile([128, 512], FP, tag="rb")
                nc.tensor.matmul(rb[:, :cl], ones_row[:], rstd[:, c0:c0 + cl],
                                 start=True, stop=True)
                mb = s_ps.tile([128, 512], FP, tag="mb")
                nc.tensor.matmul(mb[:, :cl], ones_row[:], negms[:, c0:c0 + cl],
                                 start=True, stop=True)
                for ft in range(FT):
                    nc.vector.tensor_tensor(out=cr[:, ft, c0:c0 + cl],
                                            in0=cr[:, ft, c0:c0 + cl],
                                            in1=rb[:, :cl], op=ALU.mult)
                    nc.vector.tensor_tensor(out=cr[:, ft, c0:c0 + cl],
                                            in0=cr[:, ft, c0:c0 + cl],
                                            in1=mb[:, :cl], op=ALU.add)
                    if not flags["cn1"]:
                        nc.vector.tensor_scalar(out=cr[:, ft, c0:c0 + cl],
                                                in0=cr[:, ft, c0:c0 + cl],
                                                scalar1=cng_sb[:, j, ft, :],
                                                scalar2=cnb_sb[:, j, ft, :],
                                                op0=ALU.mult, op1=ALU.add)
            # re-zero the out-of-graph halo (cores 0/7): the reference
            # zero-pads at every conv layer, and conv smears real values
            # into the halo otherwise
            for ft in range(FT):
                nc.vector.tensor_scalar_mul(out=cr[:, ft, 0:HALO],
                                            in0=cr[:, ft, 0:HALO], scalar1=ml_sb[:])
                nc.vector.tensor_scalar_mul(out=cr[:, ft, CW - HALO:CW],
                                            in0=cr[:, ft, CW - HALO:CW], scalar1=mr_sb[:])
            c_fm = cr
        cps.close()
        if dbg:
            for ft in range(FT):
                nc.sync.dma_start(out=dbg_cnn[:, ft * CW:(ft + 1) * CW],
                                  in_=c_fm[:, ft, :])

        # ---- phase 3: fused projection + attention ----
        attn = ExitStack()
        aw = attn.enter_context(tc.tile_pool(name="attnw", bufs=1))
        q_sb = aw.tile([128, HEADS, NPC], FP)
        o_fm = aw.tile([128, HEADS, NPC], FP)

        qkvtmp = ExitStack()
        qtp = qkvtmp.enter_context(tc.tile_pool(name="qkvtmp", bufs=1))
        ipw_sb = qtp.tile([128, 4, 3 * E2], FP)
        for kt in range(4):
            nc.sync.dma_start(out=ipw_sb[:, kt, :], in_=ipwT[kt])
        kvt_sb = qtp.tile([128, 8, NPC], FP)
        if not flags["ipb0"]:
            ipb_sb = qtp.tile([128, 12, 1], FP)
            nc.sync.dma_start(out=ipb_sb[:], in_=ipb_in[:])
        v_nm = qtp.tile([128, NTC, E2], FP)

        qkv_ps = ExitStack()
        q_ps = qkv_ps.enter_context(tc.tile_pool(name="q_ps", bufs=3, space="PSUM"))
        qt_ps = qkv_ps.enter_context(tc.tile_pool(name="qt_ps", bufs=2, space="PSUM"))
        # fused_fm tiles: [gnn ho_fm (2) | cnn c_fm own (2)]
        fused = [ho_fm[:, 0, :], ho_fm[:, 1, :],
                 c_fm[:, 0, HALO:HALO + NPC], c_fm[:, 1, HALO:HALO + NPC]]
        for ot in range(12):
            psq = q_ps.tile([128, NPC], FP, tag="psq")
            for kt in range(4):
                nc.tensor.matmul(psq[:], ipw_sb[:, kt, ot * 128:(ot + 1) * 128],
                                 fused[kt], start=(kt == 0), stop=(kt == 3))
            dst = q_sb[:, ot, :] if ot < 4 else kvt_sb[:, ot - 4, :]
            if flags["ipb0"]:
                nc.vector.tensor_copy(out=dst, in_=psq[:])
            else:
                nc.vector.tensor_scalar(out=dst, in0=psq[:],
                                        scalar1=ipb_sb[:, ot, :], scalar2=None,
                                        op0=ALU.add)
        # v (kvt tiles 4..7) feature-major -> node-major for AV lhsT
        for nt in range(NTC):
            for vt in range(4):
                pt = qt_ps.tile([128, 128], FP, tag="qtps")
                nc.tensor.transpose(pt[:], kvt_sb[:, 4 + vt, nt * 128:(nt + 1) * 128],
                                    ident[:])
                nc.vector.tensor_copy(out=v_nm[:, nt, vt * 128:(vt + 1) * 128], in_=pt[:])
        qkv_ps.close()
        if dbg:
            for ot in range(12):
                srcq = q_sb[:, ot, :] if ot < 4 else kvt_sb[:, ot - 4, :]
                nc.sync.dma_start(out=dbg_qkv[:, ot * NPC:(ot + 1) * NPC], in_=srcq)

        kh_outs, vh_outs = [], []
        for h in range(HEADS):
            kh_in = dram.tile([128, NPC], FP, name=f"kh_in{h}")
            nc.sync.dma_start(out=kh_in[:], in_=kvt_sb[:, h, :])
            kh_out = dram.tile([NCORES * 128, NPC], FP, name=f"kh_out{h}",
                               addr_space="Shared")
            nc.gpsimd.collective_compute("AllGather", ALU.bypass, replica_groups=RG,
                                         ins=[kh_in.opt()], outs=[kh_out.opt()])
            kh_outs.append(kh_out)
            vh_in = dram.tile([NPC, 128], FP, name=f"vh_in{h}")
            for nt in range(NTC):
                nc.sync.dma_start(out=vh_in[nt * 128:(nt + 1) * 128, :],
                                  in_=v_nm[:, nt, h * 128:(h + 1) * 128])
            vh_out = dram.tile([N, 128], FP, name=f"vh_out{h}", addr_space="Shared")
            nc.gpsimd.collective_compute("AllGather", ALU.bypass, replica_groups=RG,
                                         ins=[vh_in.opt()], outs=[vh_out.opt()])
            vh_outs.append(vh_out)
        qkvtmp.close()

        heads_es = ExitStack()
        kv_pool = heads_es.enter_context(tc.tile_pool(name="kv", bufs=2))
        pt_pool = heads_es.enter_context(tc.tile_pool(name="ptp", bufs=2))
        at_ps = ExitStack()
        st_ps = at_ps.enter_context(tc.tile_pool(name="st_ps", bufs=2, space="PSUM"))
        o_ps = at_ps.enter_context(tc.tile_pool(name="o_ps", bufs=1, space="PSUM"))
        rs_ps = at_ps.enter_context(tc.tile_pool(name="rs_ps", bufs=1, space="PSUM"))
        inv_sqrt_hd = 1.0 / float(np.sqrt(HD))
        GRP = 3
        groups = [list(range(g, min(g + GRP, NT))) for g in range(0, NT, GRP)]
        for h in range(HEADS):
            k_sb = kv_pool.tile([128, N], FP, tag="k_sb")
            for r in range(NCORES):
                nc.sync.dma_start(out=k_sb[:, r * NPC:(r + 1) * NPC],
                                  in_=kh_outs[h][r * 128:(r + 1) * 128, :])
            v_sb = kv_pool.tile([128, NT, 128], FP, tag="v_sb")
            for st in range(NT):
                nc.sync.dma_start(out=v_sb[:, st, :],
                                  in_=vh_outs[h][st * 128:(st + 1) * 128, :])
            pso = o_ps.tile([128, NPC], FP, tag="pso")
            psr = rs_ps.tile([1, NPC], FP, tag="psr")
            for grp in groups:
                pst = st_ps.tile([128, GRP * NPC], FP, tag="pst")
                for gi, s in enumerate(grp):
                    nc.tensor.matmul(pst[:, gi * NPC:(gi + 1) * NPC],
                                     k_sb[:, s * 128:(s + 1) * 128],
                                     q_sb[:, h, :], start=True, stop=True)
                p_sb = pt_pool.tile([128, GRP * NPC], FP, tag="p_sb")
                nc.scalar.activation(out=p_sb[:, :len(grp) * NPC],
                                     in_=pst[:, :len(grp) * NPC],
                                     func=AF.Exp, scale=inv_sqrt_hd)
                for gi, s in enumerate(grp):
                    rhs = p_sb[:, gi * NPC:(gi + 1) * NPC]
                    nc.tensor.matmul(pso[:], v_sb[:, s, :], rhs,
                                     start=(s == 0), stop=(s == NT - 1))
                    nc.tensor.matmul(psr[:], ones_t[:], rhs,
                                     start=(s == 0), stop=(s == NT - 1))
            rr = pt_pool.tile([1, NPC], FP, tag="rr")
            nc.vector.tensor_copy(out=rr[:], in_=psr[:])
            nc.vector.reciprocal(out=rr[:], in_=rr[:])
            rb2 = rs_ps.tile([128, NPC], FP, tag="psr")
            nc.tensor.matmul(rb2[:], ones_row[:], rr[:], start=True, stop=True)
            ou_sb = pt_pool.tile([128, NPC], FP, tag="ou_sb")
            nc.vector.tensor_copy(out=ou_sb[:], in_=pso[:])
            nc.vector.tensor_tensor(out=o_fm[:, h, :], in0=ou_sb[:], in1=rb2[:],
                                    op=ALU.mult)
        at_ps.close()
        heads_es.close()
        if dbg:
            for h in range(HEADS):
                nc.sync.dma_start(out=dbg_ofm[:, h * NPC:(h + 1) * NPC],
                                  in_=o_fm[:, h, :])

        # ---- phase 4: out_proj + layernorm + fuse head ----
        tailw = ExitStack()
        tw = tailw.enter_context(tc.tile_pool(name="tailw", bufs=1))
        ttmp = tailw.enter_context(tc.tile_pool(name="ttmp", bufs=3))
        tl_ps = ExitStack()
        p_ps = tl_ps.enter_context(tc.tile_pool(name="p_ps", bufs=2, space="PSUM"))
        a_ps = tl_ps.enter_context(tc.tile_pool(name="a_ps", bufs=1, space="PSUM"))
        opw_sb = tw.tile([128, 4, E2], FP)
        for kt in range(4):
            nc.sync.dma_start(out=opw_sb[:, kt, :], in_=opwT[kt])
        fw_sb = tw.tile([128, 4, OUT], FP)
        for kt in range(4):
            nc.sync.dma_start(out=fw_sb[:, kt, :], in_=fwT[kt])
        if not flags["opb0"]:
            opb_sb = tw.tile([128, 4, 1], FP)
            nc.sync.dma_start(out=opb_sb[:], in_=opb_in[:])
        if not flags["an1"]:
            ang_sb = tw.tile([128, 4, 1], FP)
            anb_sb = tw.tile([128, 4, 1], FP)
            nc.sync.dma_start(out=ang_sb[:], in_=ang_in[:])
            nc.sync.dma_start(out=anb_sb[:], in_=anb_in[:])
        if not flags["fb0"]:
            fb_sb = tw.tile([64, 1], FP)
            nc.sync.dma_start(out=fb_sb[:], in_=fb_in[:])

        op_sb = tw.tile([128, 4, NPC], FP)
        for ot in range(4):
            psp = p_ps.tile([128, NPC], FP, tag="psp")
            for kt in range(4):
                nc.tensor.matmul(psp[:], opw_sb[:, kt, ot * 128:(ot + 1) * 128],
                                 o_fm[:, kt, :], start=(kt == 0), stop=(kt == 3))
            if flags["opb0"]:
                nc.vector.tensor_copy(out=op_sb[:, ot, :], in_=psp[:])
            else:
                nc.vector.tensor_scalar(out=op_sb[:, ot, :], in0=psp[:],
                                        scalar1=opb_sb[:, ot, :], scalar2=None, op0=ALU.add)
        # anorm LN over 512 features (partition reduce over 4 tiles)
        pss = a_ps.tile([1, NPC], FP, tag="pssa")
        psq2 = a_ps.tile([1, NPC], FP, tag="psqa")
        sq2 = ttmp.tile([128, 4, NPC], FP, tag="sq2")
        for kt in range(4):
            nc.vector.tensor_mul(out=sq2[:, kt, :], in0=op_sb[:, kt, :], in1=op_sb[:, kt, :])
        for kt in range(4):
            nc.tensor.matmul(pss[:], ones_t[:], op_sb[:, kt, :],
                             start=(kt == 0), stop=(kt == 3))
            nc.tensor.matmul(psq2[:], ones_t[:], sq2[:, kt, :],
                             start=(kt == 0), stop=(kt == 3))
        mean2 = ttmp.tile([1, NPC], FP, tag="mean2")
        nc.vector.tensor_scalar(out=mean2[:], in0=pss[:], scalar1=1.0 / E2, scalar2=None,
                                op0=ALU.mult)
        var2 = ttmp.tile([1, NPC], FP, tag="var2")
        nc.vector.tensor_mul(out=var2[:], in0=mean2[:], in1=mean2[:])
        sq_m = ttmp.tile([1, NPC], FP, tag="sq_m")
        nc.vector.tensor_scalar(out=sq_m[:], in0=psq2[:], scalar1=1.0 / E2, scalar2=None,
                                op0=ALU.mult)
        nc.vector.tensor_tensor(out=var2[:], in0=sq_m[:], in1=var2[:], op=ALU.subtract)
        # rstd = exp(-0.5 * ln(var + eps)) (keeps us in the ln/exp ACT table set)
        rstd2 = ttmp.tile([1, NPC], FP, tag="rstd2")
        nc.scalar.activation(out=rstd2[:], in_=var2[:], func=AF.Ln, bias=eps_t[:1, :],
                             scale=1.0)
        nc.scalar.activation(out=rstd2[:], in_=rstd2[:], func=AF.Exp, scale=-0.5)
        negm2 = ttmp.tile([1, NPC], FP, tag="negm2")
        nc.vector.tensor_mul(out=negm2[:], in0=mean2[:], in1=rstd2[:])
        nc.vector.tensor_scalar(out=negm2[:], in0=negm2[:], scalar1=-1.0, scalar2=None,
                                op0=ALU.mult)
        rb3 = a_ps.tile([128, NPC], FP, tag="pssa")
        nc.tensor.matmul(rb3[:], ones_row[:], rstd2[:], start=True, stop=True)
        mb3 = a_ps.tile([128, NPC], FP, tag="psqa")
        nc.tensor.matmul(mb3[:], ones_row[:], negm2[:], start=True, stop=True)
        for kt in range(4):
            nc.vector.tensor_tensor(out=op_sb[:, kt, :], in0=op_sb[:, kt, :],
                                    in1=rb3[:], op=ALU.mult)
            nc.vector.tensor_tensor(out=op_sb[:, kt, :], in0=op_sb[:, kt, :],
                                    in1=mb3[:], op=ALU.add)
            if not flags["an1"]:
                nc.vector.tensor_scalar(out=op_sb[:, kt, :], in0=op_sb[:, kt, :],
                                        scalar1=ang_sb[:, kt, :], scalar2=anb_sb[:, kt, :],
                                        op0=ALU.mult, op1=ALU.add)
        if dbg:
            for kt in range(4):
                nc.sync.dma_start(out=dbg_opn[:, kt * NPC:(kt + 1) * NPC],
                                  in_=op_sb[:, kt, :])
        # fuse head: [64, 512] then transpose to node-major output
        psf = p_ps.tile([64, NPC], FP, tag="psf")
        for kt in range(4):
            nc.tensor.matmul(psf[:], fw_sb[:, kt, :], op_sb[:, kt, :],
                             start=(kt == 0), stop=(kt == 3))
        ff_sb = ttmp.tile([64, NPC], FP, tag="ff_sb")
        if flags["fb0"]:
            nc.vector.tensor_copy(out=ff_sb[:], in_=psf[:])
        else:
            nc.vector.tensor_scalar(out=ff_sb[:], in0=psf[:], scalar1=fb_sb[:],
                                    scalar2=None, op0=ALU.add)
        for nt in range(NTC):
            ptf = a_ps.tile([128, 64], FP, tag="ptf")
            nc.tensor.matmul(ptf[:], ff_sb[:, nt * 128:(nt + 1) * 128], ident[:64, :64],
                             is_transpose=True)
            of_sb = ttmp.tile([128, 64], FP, tag="of_sb")
            nc.vector.tensor_copy(out=of_sb[:], in_=ptf[:])
            nc.sync.dma_start(out=out_d[nt * 128:(nt + 1) * 128, :], in_=of_sb[:])
        tailw.close()
        tl_ps.close()
        attn.close()
        cnn.close()
        stack.close()

    nc.compile()
    return nc


def cr_prev_slice(c_fm, it, c0, cl):
    return c_fm[:, it, c0:c0 + cl]


def _prep(inputs):
    """Host-side shard/transform. Returns per-core in_maps + flags."""
    x = np.ascontiguousarray(np.asarray(inputs["x"], np.float32))
    ei = np.asarray(inputs["edge_index"], np.int64)
    src, dst = ei[0], ei[1]
    deg = np.bincount(dst, minlength=N).astype(np.float32)
    deg = np.maximum(deg, 1.0)
    A = np.zeros((N, N), np.float32)
    np.add.at(A, (dst, src), 1.0)
    A /= deg[:, None]

    sage_wl = np.asarray(inputs["sage_wl"], np.float32)
    sage_wr = np.asarray(inputs["sage_wr"], np.float32)
    wlT = np.ascontiguousarray(
        sage_wl.transpose(0, 2, 1).reshape(L, FT, 128, H))
    wrT = np.ascontiguousarray(
        sage_wr.transpose(0, 2, 1).reshape(L, FT, 128, H))
    conv_w = np.asarray(inputs["conv_w"], np.float32)       # [3, O, I, K]
    cwT = np.ascontiguousarray(
        conv_w.transpose(0, 3, 2, 1).reshape(3, 3, FT, 128, H))
    ipwT = np.ascontiguousarray(
        np.asarray(inputs["in_proj_w"], np.float32).T.reshape(4, 128, 3 * E2))
    opwT = np.ascontiguousarray(
        np.asarray(inputs["out_proj_w"], np.float32).T.reshape(4, 128, E2))
    fwT = np.ascontiguousarray(
        np.asarray(inputs["fuse_w"], np.float32).T.reshape(4, 128, OUT))

    bl = np.asarray(inputs["sage_bl"], np.float32)
    lng = np.asarray(inputs["ln_g"], np.float32)
    lnb = np.asarray(inputs["ln_b"], np.float32)
    cb = np.asarray(inputs["conv_b"], np.float32)
    cng = np.asarray(inputs["cnorm_g"], np.float32)
    cnb = np.asarray(inputs["cnorm_b"], np.float32)
    ipb = np.asarray(inputs["in_proj_b"], np.float32)
    opb = np.asarray(inputs["out_proj_b"], np.float32)
    ang = np.asarray(inputs["anorm_g"], np.float32)
    anb = np.asarray(inputs["anorm_b"], np.float32)
    fb = np.asarray(inputs["fuse_b"], np.float32)
    flags = {
        "bl0": bool(np.all(bl == 0)),
        "ln1": bool(np.all(lng == 1) and np.all(lnb == 0)),
        "cb0": bool(np.all(cb == 0)),
        "cn1": bool(np.all(cng == 1) and np.all(cnb == 0)),
        "ipb0": bool(np.all(ipb == 0)),
        "opb0": bool(np.all(opb == 0)),
        "an1": bool(np.all(ang == 1) and np.all(anb == 0)),
        "fb0": bool(np.all(fb == 0)),
    }

    in_maps = []
    for c in range(NCORES):
        rows = slice(c * NPC, (c + 1) * NPC)
        a_ct = np.ascontiguousarray(A[rows].T.reshape(NT, 128, NPC))
        # halo indices into cc_halo_out [8 * 256, H]: rank r rows [256r, 256r+256)
        # = [first 128-node tile; last 128-node tile] of rank r
        if c > 0:
            idx_l = (256 * (c - 1) + 128 + np.arange(HALO)).astype(np.int32)
        else:
            idx_l = np.zeros(HALO, np.int32)
        if c < NCORES - 1:
            idx_r = (256 * (c + 1) + np.arange(HALO)).astype(np.int32)
        else:
            idx_r = np.zeros(HALO, np.int32)
        m = {
            "x_full": x,
            "x_own": np.ascontiguousarray(x[rows]),
            "a_ct": a_ct,
            "wlT": wlT, "wrT": wrT, "cwT": cwT,
            "ipwT": ipwT, "opwT": opwT, "fwT": fwT,
            "idx_l": idx_l.reshape(HALO, 1),
            "idx_r": idx_r.reshape(HALO, 1),
            "mask_l": np.full((HALO, 1), 0.0 if c == 0 else 1.0, np.float32),
            "mask_r": np.full((HALO, 1), 0.0 if c == NCORES - 1 else 1.0, np.float32),
        }
        if not flags["bl0"]:
            m["bl_in"] = np.ascontiguousarray(bl.reshape(1, L, H))
        if not flags["ln1"]:
            m["lng_in"] = np.ascontiguousarray(lng.reshape(1, L - 1, H))
            m["lnb_in"] = np.ascontiguousarray(lnb.reshape(1, L - 1, H))
        if not flags["cb0"]:
            m["cb_in"] = np.ascontiguousarray(
                cb.reshape(3, FT, 128).transpose(2, 0, 1).reshape(128, 3, FT, 1))
        if not flags["cn1"]:
            m["cng_in"] = np.ascontiguousarray(
                cng.reshape(3, FT, 128).transpose(2, 0, 1).reshape(128, 3, FT, 1))
            m["cnb_in"] = np.ascontiguousarray(
                cnb.reshape(3, FT, 128).transpose(2, 0, 1).reshape(128, 3, FT, 1))
        if not flags["ipb0"]:
            m["ipb_in"] = np.ascontiguousarray(
                ipb.reshape(12, 128).T.reshape(128, 12, 1))
        if not flags["opb0"]:
            m["opb_in"] = np.ascontiguousarray(
                opb.reshape(4, 128).T.reshape(128, 4, 1))
        if not flags["an1"]:
            m["ang_in"] = np.ascontiguousarray(
                ang.reshape(4, 128).T.reshape(128, 4, 1))
            m["anb_in"] = np.ascontiguousarray(
                anb.reshape(4, 128).T.reshape(128, 4, 1))
        if not flags["fb0"]:
            m["fb_in"] = np.ascontiguousarray(fb.reshape(OUT, 1))
        in_maps.append(m)
    return in_maps, flags


last_exec_time_ns = None


last_debug = None


def kernel(**inputs) -> np.ndarray:
    global last_exec_time_ns, last_debug
    import os
    dbg = os.environ.get("KERNEL_DEBUG", "0") == "1"
    in_maps, flags = _prep(inputs)
    key = (dbg,) + tuple(sorted(flags.items()))
    if key not in _CACHE:
        _CACHE[key] = _build(flags, dbg=dbg)
    nc = _CACHE[key]
    trace = os.environ.get("KERNEL_TRACE", "0") == "1"
    res = run_bass_kernel_spmd(nc, in_maps, core_ids=list(range(NCORES)),
                               trace=trace)
    last_exec_time_ns = res.exec_time_ns
    if dbg:
        last_debug = res.results
    out = np.concatenate([r["out"] for r in res.results], axis=0)
    return out.astype(np.float32)


# revision 19
# speedup vs baseline: 2.0981x; 1.9643x over previous
# Trainium2 Bass kernel for AdvancedHybridHOIGNN (6x SAGEConv + 3x Conv1d + MHA + fuse).
#
# Sharding: 4096 nodes split 512/core across 8 cores. The SAGE neighbor
# aggregation is dense matmuls against a host-built adjacency-count slice
# (bf16-exact counts; the 1/deg scaling is folded in after the wl matmul as a
# per-partition scale). Node features are replicated each layer via AllGather
# (bf16 payloads). The CNN branch needs a 128-node halo, exchanged with a
# small first/last-tile AllGather and fetched by indirect DMA with per-core
# index data so the single SPMD program stays uniform. Attention is
# sequence-parallel over queries with AllGathered K/V.
#
# fp32 matmuls execute as TWO passes on trn2 (FP32HI/LO), so all matmul
# operands are bf16; accumulation stays fp32 in PSUM, and LayerNorm/residual
# paths stay fp32.

import sys

sys.path.insert(0, "/opt/trn_rl_repo")

import ml_dtypes
import numpy as np

import concourse.bass as bass
import concourse.mybir as mybir
import concourse.tile as tile
from concourse import bacc
from concourse.bass_utils import run_bass_kernel_spmd
from concourse.masks import make_identity

FP = mybir.dt.float32
BF = mybir.dt.bfloat16
AF = mybir.ActivationFunctionType
ALU = mybir.AluOpType
BF_NP = ml_dtypes.bfloat16

N, H, OUT, L = 4096, 256, 64, 6
NCORES = 8
NPC = N // NCORES          # 512 nodes per core
NT = N // 128              # 32 node tiles globally
NTC = NPC // 128           # 4 node tiles per core
FT = H // 128              # 2 feature tiles
E2 = 2 * H                 # 512
HEADS, HD = 4, 128
HALO = 128                 # halo width for the conv branch (needs only 3)
CW = NPC + 2 * HALO        # 768: conv working width per core
EPS = 1e-5

_CACHE = {}


def _build(flags, dbg=False):
    nc = bacc.Bacc("TRN2", target_bir_lowering=False, debug=False,
                   num_devices=NCORES)
    RG = [list(range(NCORES))]

    # ---------------- kernel I/O ----------------
    x_bf = nc.dram_tensor("x_bf", [N, H], BF, kind="ExternalInput")
    x_own = nc.dram_tensor("x_own", [NPC, H], FP, kind="ExternalInput")
    a_ct = nc.dram_tensor("a_ct", [NT, 128, NPC], BF, kind="ExternalInput")
    invdeg = nc.dram_tensor("invdeg", [128, NTC, 1], FP, kind="ExternalInput")
    wlT = nc.dram_tensor("wlT", [L, FT, 128, H], BF, kind="ExternalInput")
    wrT = nc.dram_tensor("wrT", [L, FT, 128, H], BF, kind="ExternalInput")
    cwT = nc.dram_tensor("cwT", [3, 3, FT, 128, H], BF, kind="ExternalInput")
    ipwT = nc.dram_tensor("ipwT", [4, 128, 3 * E2], BF, kind="ExternalInput")
    opwT = nc.dram_tensor("opwT", [4, 128, E2], BF, kind="ExternalInput")
    fwT = nc.dram_tensor("fwT", [4, 128, OUT], BF, kind="ExternalInput")
    idx_l = nc.dram_tensor("idx_l", [HALO, 1], mybir.dt.int32, kind="ExternalInput")
    idx_r = nc.dram_tensor("idx_r", [HALO, 1], mybir.dt.int32, kind="ExternalInput")
    mask_l = nc.dram_tensor("mask_l", [HALO, 1], FP, kind="ExternalInput")
    mask_r = nc.dram_tensor("mask_r", [HALO, 1], FP, kind="ExternalInput")
    if not flags["bl0"]:
        bl_in = nc.dram_tensor("bl_in", [1, L, H], FP, kind="ExternalInput")
    if not flags["ln1"]:
        lng_in = nc.dram_tensor("lng_in", [1, L - 1, H], FP, kind="ExternalInput")
        lnb_in = nc.dram_tensor("lnb_in", [1, L - 1, H], FP, kind="ExternalInput")
    if not flags["cb0"]:
        cb_in = nc.dram_tensor("cb_in", [128, 3, FT, 1], FP, kind="ExternalInput")
    if not flags["cn1"]:
        cng_in = nc.dram_tensor("cng_in", [128, 3, FT, 1], FP, kind="ExternalInput")
        cnb_in = nc.dram_tensor("cnb_in", [128, 3, FT, 1], FP, kind="ExternalInput")
    if not flags["ipb0"]:
        ipb_in = nc.dram_tensor("ipb_in", [128, 12, 1], FP, kind="ExternalInput")
    if not flags["opb0"]:
        opb_in = nc.dram_tensor("opb_in", [128, 4, 1], FP, kind="ExternalInput")
    if not flags["an1"]:
        ang_in = nc.dram_tensor("ang_in", [128, 4, 1], FP, kind="ExternalInput")
        anb_in = nc.dram_tensor("anb_in", [128, 4, 1], FP, kind="ExternalInput")
    if not flags["fb0"]:
        fb_in = nc.dram_tensor("fb_in", [64, 1], FP, kind="ExternalInput")
    out_d = nc.dram_tensor("out", [NPC, OUT], FP, kind="ExternalOutput")
    if dbg:
        dbg_gnn = nc.dram_tensor("dbg_gnn", [NPC, H], FP, kind="ExternalOutput")
        dbg_cnn = nc.dram_tensor("dbg_cnn", [128, FT * CW], BF, kind="ExternalOutput")
        dbg_qkv = nc.dram_tensor("dbg_qkv", [128, 12 * NPC], BF, kind="ExternalOutput")
        dbg_ofm = nc.dram_tensor("dbg_ofm", [128, HEADS * NPC], BF, kind="ExternalOutput")
        dbg_opn = nc.dram_tensor("dbg_opn", [128, 4 * NPC], FP, kind="ExternalOutput")

    from contextlib import ExitStack

    with tile.TileContext(nc) as tc:
        stack = ExitStack()
        # long-lived pools on the right SBUF side; phase pools stack LIFO left
        singles = stack.enter_context(tc.tile_pool(name="singles", bufs=1, side="right"))
        dram = stack.enter_context(tc.tile_pool(name="dram", bufs=1, space="DRAM"))

        ident = singles.tile([128, 128], FP)
        make_identity(nc, ident[:])
        ident_bf = singles.tile([128, 128], BF)
        make_identity(nc, ident_bf[:])
        eps_t = singles.tile([128, 1], FP)
        nc.vector.memset(eps_t[:], EPS)
        ones_bf = singles.tile([128, 1], BF)
        nc.vector.memset(ones_bf[:], 1.0)
        ones_f = singles.tile([128, 1], FP)
        nc.vector.memset(ones_f[:], 1.0)
        ones_row = singles.tile([1, 128], FP)
        nc.vector.memset(ones_row[:], 1.0)
        invdeg_sb = singles.tile([128, NTC, 1], FP)
        nc.sync.dma_start(out=invdeg_sb[:], in_=invdeg[:])

        # ---- phase 1: SAGE layers ----
        sagew = ExitStack()
        sw = sagew.enter_context(tc.tile_pool(name="sagew", bufs=1))
        act_sb = sw.tile([128, NT, NPC], BF)          # adjacency counts, resident
        for s in range(NT):
            nc.sync.dma_start(out=act_sb[:, s, :], in_=a_ct[s])
        wl_sb = sw.tile([128, L, FT, H], BF)
        wr_sb = sw.tile([128, L, FT, H], BF)
        for i in range(L):
            for ft in range(FT):
                nc.sync.dma_start(out=wl_sb[:, i, ft, :], in_=wlT[i, ft])
                nc.sync.dma_start(out=wr_sb[:, i, ft, :], in_=wrT[i, ft])
        if not flags["bl0"]:
            bl_sb = sw.tile([128, L, H], FP)
            nc.gpsimd.dma_start(out=bl_sb[:], in_=bl_in[:].to_broadcast([128, L, H]))
        if not flags["ln1"]:
            lng_sb = sw.tile([128, L - 1, H], FP)
            lnb_sb = sw.tile([128, L - 1, H], FP)
            nc.gpsimd.dma_start(out=lng_sb[:], in_=lng_in[:].to_broadcast([128, L - 1, H]))
            nc.gpsimd.dma_start(out=lnb_sb[:], in_=lnb_in[:].to_broadcast([128, L - 1, H]))

        ho_pool = stack.enter_context(tc.tile_pool(name="ho", bufs=2, side="right"))
        hs_pool = ExitStack()
        hstream = hs_pool.enter_context(tc.tile_pool(name="hstream", bufs=8))
        sage_ps = ExitStack()
        agg_ps = sage_ps.enter_context(tc.tile_pool(name="agg_ps", bufs=1, space="PSUM"))
        z_ps = sage_ps.enter_context(tc.tile_pool(name="z_ps", bufs=2, space="PSUM"))
        t_ps = sage_ps.enter_context(tc.tile_pool(name="t_ps", bufs=2, space="PSUM"))
        sage_tmp = ExitStack()
        stmp = sage_tmp.enter_context(tc.tile_pool(name="stmp", bufs=4))
        aggp = sage_tmp.enter_context(tc.tile_pool(name="aggsb", bufs=2))

        # initial own-slice: node-major fp32 + feature-major bf16
        ho_nm = ho_pool.tile([128, NTC, H], FP, name="ho_nm0", tag="ho_nm")
        for nt in range(NTC):
            nc.sync.dma_start(out=ho_nm[:, nt, :], in_=x_own[nt * 128:(nt + 1) * 128, :])
        ho_fm = ho_pool.tile([128, FT, NPC], BF, name="ho_fm0", tag="ho_fm")
        for nt in range(NTC):
            for ft in range(FT):
                pt = t_ps.tile([128, 128], FP, tag="tps")
                nc.tensor.transpose(pt[:], ho_nm[:, nt, ft * 128:(ft + 1) * 128], ident[:])
                nc.vector.tensor_copy(out=ho_fm[:, ft, nt * 128:(nt + 1) * 128], in_=pt[:])

        cc_outs = []
        cc_halo_out = None
        for i in range(L):
            hsrc = x_bf if i == 0 else cc_outs[i - 1]
            # agg_cnt_fm[f, d] = sum_s h[s, f] * count[s, d]
            psa = [agg_ps.tile([128, NPC], FP, name=f"psa{i}_{ft}", tag=f"psa{ft}")
                   for ft in range(FT)]
            for s in range(NT):
                hk = hstream.tile([128, H], BF, tag="hk")
                nc.sync.dma_start(out=hk[:], in_=hsrc[s * 128:(s + 1) * 128, :])
                for ft in range(FT):
                    nc.tensor.matmul(psa[ft][:], hk[:, ft * 128:(ft + 1) * 128],
                                     act_sb[:, s, :], start=(s == 0), stop=(s == NT - 1))
            agg_fm = aggp.tile([128, FT, NPC], BF, tag="agg_fm")
            for ft in range(FT):
                nc.vector.tensor_copy(out=agg_fm[:, ft, :], in_=psa[ft][:])

            # z[n, o] = (agg_cnt @ wl.T) * invdeg + h @ wr.T   (node-major out)
            ho_nm_new = ho_pool.tile([128, NTC, H], FP, name=f"ho_nm{i + 1}", tag="ho_nm")
            ho_bf_new = ho_pool.tile([128, NTC, H], BF, name=f"ho_bf{i + 1}", tag="ho_bf")
            ho_fm_new = ho_pool.tile([128, FT, NPC], BF, name=f"ho_fm{i + 1}", tag="ho_fm")
            for nt in range(NTC):
                ns = slice(nt * 128, (nt + 1) * 128)
                psza = z_ps.tile([128, H], FP, tag="psza")
                for ft in range(FT):
                    nc.tensor.matmul(psza[:], agg_fm[:, ft, ns], wl_sb[:, i, ft, :],
                                     start=(ft == 0), stop=(ft == FT - 1))
                pszr = z_ps.tile([128, H], FP, tag="pszr")
                for ft in range(FT):
                    nc.tensor.matmul(pszr[:], ho_fm[:, ft, ns], wr_sb[:, i, ft, :],
                                     start=(ft == 0), stop=(ft == FT - 1))
                zt = stmp.tile([128, H], FP, tag="zt")
                nc.vector.tensor_scalar(out=zt[:], in0=psza[:],
                                        scalar1=invdeg_sb[:, nt, :], scalar2=None,
                                        op0=ALU.mult)
                z_sb = stmp.tile([128, H], FP, tag="z_sb")
                nc.vector.tensor_tensor(out=z_sb[:], in0=zt[:], in1=pszr[:], op=ALU.add)
                if not flags["bl0"]:
                    nc.vector.tensor_tensor(out=z_sb[:], in0=z_sb[:],
                                            in1=bl_sb[:, i, :], op=ALU.add)
                if i < L - 1:
                    stat = stmp.tile([128, 6], FP, tag="stat")
                    nc.vector.bn_stats(out=stat[:], in_=z_sb[:])
                    mv = stmp.tile([128, 2], FP, tag="mv")
                    nc.vector.bn_aggr(out=mv[:], in_=stat[:])
                    sd = stmp.tile([128, 1], FP, tag="sd")
                    nc.scalar.activation(out=sd[:], in_=mv[:, 1:2], func=AF.Sqrt,
                                         bias=eps_t[:], scale=1.0)
                    nc.vector.reciprocal(out=sd[:], in_=sd[:])
                    zn = stmp.tile([128, H], FP, tag="zn")
                    nc.vector.tensor_scalar(out=zn[:], in0=z_sb[:], scalar1=mv[:, 0:1],
                                            scalar2=sd[:], op0=ALU.subtract, op1=ALU.mult)
                    if not flags["ln1"]:
                        nc.vector.tensor_tensor(out=zn[:], in0=zn[:],
                                                in1=lng_sb[:, i, :], op=ALU.mult)
                        nc.vector.tensor_tensor(out=zn[:], in0=zn[:],
                                                in1=lnb_sb[:, i, :], op=ALU.add)
                else:
                    zn = z_sb
                zr = stmp.tile([128, H], FP, tag="zr")
                nc.scalar.activation(out=zr[:], in_=zn[:], func=AF.Relu)
                nc.vector.tensor_add(out=ho_nm_new[:, nt, :], in0=zr[:], in1=ho_nm[:, nt, :])
                nc.vector.tensor_copy(out=ho_bf_new[:, nt, :], in_=ho_nm_new[:, nt, :])
                for ft in range(FT):
                    pt = t_ps.tile([128, 128], FP, tag="tps")
                    nc.tensor.transpose(pt[:], ho_nm_new[:, nt, ft * 128:(ft + 1) * 128],
                                        ident[:])
                    nc.vector.tensor_copy(out=ho_fm_new[:, ft, ns], in_=pt[:])
            ho_nm, ho_fm, ho_bf = ho_nm_new, ho_fm_new, ho_bf_new

            if i < L - 1:
                cc_in = dram.tile([NPC, H], BF, name=f"cc_in{i}")
                for nt in range(NTC):
                    nc.sync.dma_start(out=cc_in[nt * 128:(nt + 1) * 128, :],
                                      in_=ho_bf[:, nt, :])
                cc_out = dram.tile([N, H], BF, name=f"cc_out{i}", addr_space="Shared")
                nc.gpsimd.collective_compute("AllGather", ALU.bypass, replica_groups=RG,
                                             ins=[cc_in.opt()], outs=[cc_out.opt()])
                cc_outs.append(cc_out)
            else:
                # last layer: only the conv halo needs neighbours
                cc_halo_in = dram.tile([2 * 128, H], BF, name="cc_halo_in")
                nc.sync.dma_start(out=cc_halo_in[0:128, :], in_=ho_bf[:, 0, :])
                nc.sync.dma_start(out=cc_halo_in[128:256, :], in_=ho_bf[:, NTC - 1, :])
                cc_halo_out = dram.tile([NCORES * 2 * 128, H], BF, name="cc_halo_out",
                                        addr_space="Shared")
                nc.gpsimd.collective_compute("AllGather", ALU.bypass, replica_groups=RG,
                                             ins=[cc_halo_in.opt()],
                                             outs=[cc_halo_out.opt()])

        sage_tmp.close()
        hs_pool.close()
        sagew.close()
        sage_ps.close()
        if dbg:
            for nt in range(NTC):
                nc.sync.dma_start(out=dbg_gnn[nt * 128:(nt + 1) * 128, :],
                                  in_=ho_nm[:, nt, :])

        # ---- phase 2: CNN branch (feature-major, nodes on free axis) ----
        cnn = ExitStack()
        cw_pool = cnn.enter_context(tc.tile_pool(name="cnnw", bufs=1))
        cfm_pool = cnn.enter_context(tc.tile_pool(name="cfm", bufs=2))
        ctmp = cnn.enter_context(tc.tile_pool(name="ctmp", bufs=1))
        cps = ExitStack()
        c_ps = cps.enter_context(tc.tile_pool(name="c_ps", bufs=2, space="PSUM"))
        s_ps = cps.enter_context(tc.tile_pool(name="s_ps", bufs=1, space="PSUM"))
        ctp_es = ExitStack()
        ct_ps = ctp_es.enter_context(tc.tile_pool(name="ct_ps", bufs=1, space="PSUM"))

        cw_sb = cw_pool.tile([128, 3, 3, FT, H], BF)
        for j in range(3):
            for k in range(3):
                for ft in range(FT):
                    nc.sync.dma_start(out=cw_sb[:, j, k, ft, :], in_=cwT[j, k, ft])
        if not flags["cb0"]:
            cb_sb = cw_pool.tile([128, 3, FT, 1], FP)
            nc.sync.dma_start(out=cb_sb[:], in_=cb_in[:])
        if not flags["cn1"]:
            cng_sb = cw_pool.tile([128, 3, FT, 1], FP)
            cnb_sb = cw_pool.tile([128, 3, FT, 1], FP)
            nc.sync.dma_start(out=cng_sb[:], in_=cng_in[:])
            nc.sync.dma_start(out=cnb_sb[:], in_=cnb_in[:])

        il_sb = cw_pool.tile([HALO, 1], mybir.dt.int32)
        ir_sb = cw_pool.tile([HALO, 1], mybir.dt.int32)
        ml_sb = cw_pool.tile([HALO, 1], FP)
        mr_sb = cw_pool.tile([HALO, 1], FP)
        nc.sync.dma_start(out=il_sb[:], in_=idx_l[:])
        nc.sync.dma_start(out=ir_sb[:], in_=idx_r[:])
        nc.sync.dma_start(out=ml_sb[:], in_=mask_l[:])
        nc.sync.dma_start(out=mr_sb[:], in_=mask_r[:])
        halo_l = ctmp.tile([HALO, H], BF, tag="halo")
        nc.gpsimd.indirect_dma_start(
            out=halo_l[:], out_offset=None, in_=cc_halo_out[:],
            in_offset=bass.IndirectOffsetOnAxis(ap=il_sb[:, :1], axis=0))
        nc.vector.tensor_scalar_mul(out=halo_l[:], in0=halo_l[:], scalar1=ml_sb[:])
        halo_r = ctmp.tile([HALO, H], BF, tag="halo")
        nc.gpsimd.indirect_dma_start(
            out=halo_r[:], out_offset=None, in_=cc_halo_out[:],
            in_offset=bass.IndirectOffsetOnAxis(ap=ir_sb[:, :1], axis=0))
        nc.vector.tensor_scalar_mul(out=halo_r[:], in0=halo_r[:], scalar1=mr_sb[:])

        c_fm = cfm_pool.tile([128, FT, CW], BF, tag="c_fm", name="c_fm_in")
        for w in range(6):
            for ft in range(FT):
                if w == 0 or w == 5:
                    hsrc2 = halo_l if w == 0 else halo_r
                    ptb = ct_ps.tile([128, 128], BF, tag="ctpsb")
                    nc.tensor.transpose(ptb[:], hsrc2[:, ft * 128:(ft + 1) * 128],
                                        ident_bf[:])
                    nc.vector.tensor_copy(out=c_fm[:, ft, w * 128:(w + 1) * 128],
                                          in_=ptb[:])
                else:
                    pt = ct_ps.tile([128, 128], FP, tag="ctps")
                    nc.tensor.transpose(pt[:], ho_nm[:, w - 1, ft * 128:(ft + 1) * 128],
                                        ident[:])
                    nc.vector.tensor_copy(out=c_fm[:, ft, w * 128:(w + 1) * 128],
                                          in_=pt[:])
        ctp_es.close()

        # conv layers: compute output cols [1, CW-1)
        chunks = [(1, 512), (513, CW - 1 - 513)]
        W = CW - 2
        for j in range(3):
            cr = cfm_pool.tile([128, FT, CW], BF, tag="c_fm", name=f"c_fm{j}")
            for ft in range(FT):  # guard stale edge cols
                nc.vector.memset(cr[:, ft, 0:1], 0.0)
                nc.vector.memset(cr[:, ft, CW - 1:CW], 0.0)
            for ot in range(FT):
                for (c0, cl) in chunks:
                    psc = c_ps.tile([128, 512], FP, tag="psc")
                    first = True
                    for k in range(3):
                        for it in range(FT):
                            nc.tensor.matmul(
                                psc[:, :cl],
                                cw_sb[:, j, k, it, ot * 128:(ot + 1) * 128],
                                c_fm[:, it, c0 + k - 1:c0 + k - 1 + cl],
                                start=first, stop=(k == 2 and it == FT - 1))
                            first = False
                    if flags["cb0"]:
                        nc.scalar.activation(out=cr[:, ot, c0:c0 + cl], in_=psc[:, :cl],
                                             func=AF.Relu)
                    else:
                        nc.scalar.activation(out=cr[:, ot, c0:c0 + cl], in_=psc[:, :cl],
                                             func=AF.Relu, bias=cb_sb[:, j, ot, :],
                                             scale=1.0)
            # channel LayerNorm per node (partition reduce via ones-matmul)
            sums = ctmp.tile([1, CW], FP, tag="sums")
            sumsq = ctmp.tile([1, CW], FP, tag="sumsq")
            sqt = ctmp.tile([128, FT, CW], BF, tag="sqt")
            for ft in range(FT):
                nc.vector.tensor_mul(out=sqt[:, ft, 1:1 + W], in0=cr[:, ft, 1:1 + W],
                                     in1=cr[:, ft, 1:1 + W])
            for (c0, cl) in chunks:
                pss = s_ps.tile([1, 512], FP, tag="pss")
                psq = s_ps.tile([1, 512], FP, tag="psq")
                for ft in range(FT):
                    nc.tensor.matmul(pss[:, :cl], ones_bf[:], cr[:, ft, c0:c0 + cl],
                                     start=(ft == 0), stop=(ft == FT - 1))
                    nc.tensor.matmul(psq[:, :cl], ones_bf[:], sqt[:, ft, c0:c0 + cl],
                                     start=(ft == 0), stop=(ft == FT - 1))
                nc.vector.tensor_copy(out=sums[:, c0:c0 + cl], in_=pss[:, :cl])
                nc.vector.tensor_copy(out=sumsq[:, c0:c0 + cl], in_=psq[:, :cl])
            mean = ctmp.tile([1, CW], FP, tag="mean")
            nc.vector.tensor_scalar(out=mean[:, 1:1 + W], in0=sums[:, 1:1 + W],
                                    scalar1=1.0 / H, scalar2=None, op0=ALU.mult)
            var = ctmp.tile([1, CW], FP, tag="var")
            nc.vector.tensor_mul(out=var[:, 1:1 + W], in0=mean[:, 1:1 + W],
                                 in1=mean[:, 1:1 + W])
            nc.vector.tensor_scalar(out=sumsq[:, 1:1 + W], in0=sumsq[:, 1:1 + W],
                                    scalar1=1.0 / H, scalar2=None, op0=ALU.mult)
            nc.vector.tensor_tensor(out=var[:, 1:1 + W], in0=sumsq[:, 1:1 + W],
                                    in1=var[:, 1:1 + W], op=ALU.subtract)
            # broadcast mean/var to all partitions, then rstd = 1/sqrt(var+eps)
            for (c0, cl) in chunks:
                mb = s_ps.tile([128, 512], FP, tag="mb")
                nc.tensor.matmul(mb[:, :cl], ones_row[:], mean[:, c0:c0 + cl],
                                 start=True, stop=True)
                vb = s_ps.tile([128, 512], FP, tag="vb")
                nc.tensor.matmul(vb[:, :cl], ones_row[:], var[:, c0:c0 + cl],
                                 start=True, stop=True)
                rstd = ctmp.tile([128, 512], FP, tag="rstd")
                nc.scalar.activation(out=rstd[:, :cl], in_=vb[:, :cl], func=AF.Sqrt,
                                     bias=eps_t[:], scale=1.0)
                nc.vector.reciprocal(out=rstd[:, :cl], in_=rstd[:, :cl])
                for ft in range(FT):
                    cen = ctmp.tile([128, 512], FP, tag="cen")
                    nc.vector.tensor_tensor(out=cen[:, :cl], in0=cr[:, ft, c0:c0 + cl],
                                            in1=mb[:, :cl], op=ALU.subtract)
                    nc.vector.tensor_tensor(out=cr[:, ft, c0:c0 + cl], in0=cen[:, :cl],
                                            in1=rstd[:, :cl], op=ALU.mult)
                    if not flags["cn1"]:
                        nc.vector.tensor_scalar(out=cr[:, ft, c0:c0 + cl],
                                                in0=cr[:, ft, c0:c0 + cl],
                                                scalar1=cng_sb[:, j, ft, :],
                                                scalar2=cnb_sb[:, j, ft, :],
                                                op0=ALU.mult, op1=ALU.add)
            # re-zero the out-of-graph halo (cores 0/7): the reference
            # zero-pads at every conv layer
            for ft in range(FT):
                nc.vector.tensor_scalar_mul(out=cr[:, ft, 0:HALO],
                                            in0=cr[:, ft, 0:HALO], scalar1=ml_sb[:])
                nc.vector.tensor_scalar_mul(out=cr[:, ft, CW - HALO:CW],
                                            in0=cr[:, ft, CW - HALO:CW], scalar1=mr_sb[:])
            c_fm = cr
        cps.close()
        if dbg:
            for ft in range(FT):
                nc.sync.dma_start(out=dbg_cnn[:, ft * CW:(ft + 1) * CW],
                                  in_=c_fm[:, ft, :])

        # ---- phase 3: fused projection + attention ----
        attn = ExitStack()
        aw = attn.enter_context(tc.tile_pool(name="attnw", bufs=1))
        q_sb = aw.tile([128, HEADS, NPC], BF)
        o_fm = aw.tile([128, HEADS, NPC], BF)

        qkvtmp = ExitStack()
        qtp = qkvtmp.enter_context(tc.tile_pool(name="qkvtmp", bufs=1))
        ipw_sb = qtp.tile([128, 4, 3 * E2], BF)
        for kt in range(4):
            nc.sync.dma_start(out=ipw_sb[:, kt, :], in_=ipwT[kt])
        kvt_sb = qtp.tile([128, 8, NPC], BF)
        if not flags["ipb0"]:
            ipb_sb = qtp.tile([128, 12, 1], FP)
            nc.sync.dma_start(out=ipb_sb[:], in_=ipb_in[:])
        v_nm = qtp.tile([128, NTC, E2], BF)

        qkv_ps = ExitStack()
        q_ps = qkv_ps.enter_context(tc.tile_pool(name="q_ps", bufs=3, space="PSUM"))
        qt_ps = qkv_ps.enter_context(tc.tile_pool(name="qt_ps", bufs=2, space="PSUM"))
        fused = [ho_fm[:, 0, :], ho_fm[:, 1, :],
                 c_fm[:, 0, HALO:HALO + NPC], c_fm[:, 1, HALO:HALO + NPC]]
        for ot in range(12):
            psq = q_ps.tile([128, NPC], FP, tag="psq")
            for kt in range(4):
                nc.tensor.matmul(psq[:], ipw_sb[:, kt, ot * 128:(ot + 1) * 128],
                                 fused[kt], start=(kt == 0), stop=(kt == 3))
            dst = q_sb[:, ot, :] if ot < 4 else kvt_sb[:, ot - 4, :]
            if flags["ipb0"]:
                nc.vector.tensor_copy(out=dst, in_=psq[:])
            else:
                nc.vector.tensor_scalar(out=dst, in0=psq[:],
                                        scalar1=ipb_sb[:, ot, :], scalar2=None,
                                        op0=ALU.add)
        # k AllGather fires as soon as k tiles are staged (overlaps v transposes)
        k_in = dram.tile([E2, NPC], BF, name="k_in")
        for kt in range(4):
            nc.sync.dma_start(out=k_in[kt * 128:(kt + 1) * 128, :], in_=kvt_sb[:, kt, :])
        k_out = dram.tile([NCORES * E2, NPC], BF, name="k_out", addr_space="Shared")
        nc.gpsimd.collective_compute("AllGather", ALU.bypass, replica_groups=RG,
                                     ins=[k_in.opt()], outs=[k_out.opt()])
        # v: feature-major -> node-major, then AllGather
        for nt in range(NTC):
            for vt in range(4):
                ptb = qt_ps.tile([128, 128], BF, tag="qtps")
                nc.tensor.transpose(ptb[:], kvt_sb[:, 4 + vt, nt * 128:(nt + 1) * 128],
                                    ident_bf[:])
                nc.vector.tensor_copy(out=v_nm[:, nt, vt * 128:(vt + 1) * 128], in_=ptb[:])
        qkv_ps.close()
        if dbg:
            for ot in range(12):
                srcq = q_sb[:, ot, :] if ot < 4 else kvt_sb[:, ot - 4, :]
                nc.sync.dma_start(out=dbg_qkv[:, ot * NPC:(ot + 1) * NPC], in_=srcq)
        v_in = dram.tile([NPC, E2], BF, name="v_in")
        for nt in range(NTC):
            nc.sync.dma_start(out=v_in[nt * 128:(nt + 1) * 128, :], in_=v_nm[:, nt, :])
        v_out = dram.tile([N, E2], BF, name="v_out", addr_space="Shared")
        nc.gpsimd.collective_compute("AllGather", ALU.bypass, replica_groups=RG,
                                     ins=[v_in.opt()], outs=[v_out.opt()])
        qkvtmp.close()

        heads_es = ExitStack()
        kv_pool = heads_es.enter_context(tc.tile_pool(name="kv", bufs=2))
        pt_pool = heads_es.enter_context(tc.tile_pool(name="ptp", bufs=2))
        at_ps = ExitStack()
        st_ps = at_ps.enter_context(tc.tile_pool(name="st_ps", bufs=2, space="PSUM"))
        o_ps = at_ps.enter_context(tc.tile_pool(name="o_ps", bufs=1, space="PSUM"))
        rs_ps = at_ps.enter_context(tc.tile_pool(name="rs_ps", bufs=1, space="PSUM"))
        inv_sqrt_hd = 1.0 / float(np.sqrt(HD))
        GRP = 3
        groups = [list(range(g, min(g + GRP, NT))) for g in range(0, NT, GRP)]
        for h in range(HEADS):
            k_sb = kv_pool.tile([128, N], BF, tag="k_sb")
            for r in range(NCORES):
                nc.sync.dma_start(out=k_sb[:, r * NPC:(r + 1) * NPC],
                                  in_=k_out[r * E2 + h * 128:r * E2 + (h + 1) * 128, :])
            v_sb = kv_pool.tile([128, NT, 128], BF, tag="v_sb")
            for st in range(NT):
                nc.sync.dma_start(out=v_sb[:, st, :],
                                  in_=v_out[st * 128:(st + 1) * 128,
                                            h * 128:(h + 1) * 128])
            pso = o_ps.tile([128, NPC], FP, tag="pso")
            psr = rs_ps.tile([1, NPC], FP, tag="psr")
            for grp in groups:
                pst = st_ps.tile([128, GRP * NPC], FP, tag="pst")
                for gi, s in enumerate(grp):
                    nc.tensor.matmul(pst[:, gi * NPC:(gi + 1) * NPC],
                                     k_sb[:, s * 128:(s + 1) * 128],
                                     q_sb[:, h, :], start=True, stop=True)
                p_sb = pt_pool.tile([128, GRP * NPC], BF, tag="p_sb")
                nc.scalar.activation(out=p_sb[:, :len(grp) * NPC],
                                     in_=pst[:, :len(grp) * NPC],
                                     func=AF.Exp, scale=inv_sqrt_hd)
                for gi, s in enumerate(grp):
                    rhs = p_sb[:, gi * NPC:(gi + 1) * NPC]
                    nc.tensor.matmul(pso[:], v_sb[:, s, :], rhs,
                                     start=(s == 0), stop=(s == NT - 1))
                    nc.tensor.matmul(psr[:], ones_bf[:], rhs,
                                     start=(s == 0), stop=(s == NT - 1))
            rs_sb = pt_pool.tile([1, NPC], FP, tag="rs_sb")
            nc.vector.tensor_copy(out=rs_sb[:], in_=psr[:])
            rb2 = rs_ps.tile([128, NPC], FP, tag="psr")
            nc.tensor.matmul(rb2[:], ones_row[:], rs_sb[:], start=True, stop=True)
            rr2 = pt_pool.tile([128, NPC], FP, tag="rr2")
            nc.vector.reciprocal(out=rr2[:], in_=rb2[:])
            nc.vector.tensor_tensor(out=o_fm[:, h, :], in0=rr2[:], in1=pso[:],
                                    op=ALU.mult)
        at_ps.close()
        heads_es.close()
        if dbg:
            for h in range(HEADS):
                nc.sync.dma_start(out=dbg_ofm[:, h * NPC:(h + 1) * NPC],
                                  in_=o_fm[:, h, :])

        # ---- phase 4: out_proj + layernorm + fuse head ----
        tailw = ExitStack()
        tw = tailw.enter_context(tc.tile_pool(name="tailw", bufs=1))
        ttmp = tailw.enter_context(tc.tile_pool(name="ttmp", bufs=1))
        tl_ps = ExitStack()
        p_ps = tl_ps.enter_context(tc.tile_pool(name="p_ps", bufs=2, space="PSUM"))
        a_ps = tl_ps.enter_context(tc.tile_pool(name="a_ps", bufs=1, space="PSUM"))
        opw_sb = tw.tile([128, 4, E2], BF)
        for kt in range(4):
            nc.sync.dma_start(out=opw_sb[:, kt, :], in_=opwT[kt])
        fw_sb = tw.tile([128, 4, OUT], BF)
        for kt in range(4):
            nc.sync.dma_start(out=fw_sb[:, kt, :], in_=fwT[kt])
        if not flags["opb0"]:
            opb_sb = tw.tile([128, 4, 1], FP)
            nc.sync.dma_start(out=opb_sb[:], in_=opb_in[:])
        if not flags["an1"]:
            ang_sb = tw.tile([128, 4, 1], FP)
            anb_sb = tw.tile([128, 4, 1], FP)
            nc.sync.dma_start(out=ang_sb[:], in_=ang_in[:])
            nc.sync.dma_start(out=anb_sb[:], in_=anb_in[:])
        if not flags["fb0"]:
            fb_sb = tw.tile([64, 1], FP)
            nc.sync.dma_start(out=fb_sb[:], in_=fb_in[:])

        op_sb = tw.tile([128, 4, NPC], FP)
        for ot in range(4):
            psp = p_ps.tile([128, NPC], FP, tag="psp")
            for kt in range(4):
                nc.tensor.matmul(psp[:], opw_sb[:, kt, ot * 128:(ot + 1) * 128],
                                 o_fm[:, kt, :], start=(kt == 0), stop=(kt == 3))
            if flags["opb0"]:
                nc.vector.tensor_copy(out=op_sb[:, ot, :], in_=psp[:])
            else:
                nc.vector.tensor_scalar(out=op_sb[:, ot, :], in0=psp[:],
                                        scalar1=opb_sb[:, ot, :], scalar2=None,
                                        op0=ALU.add)
        # anorm LN over 512 features (partition reduce via ones-matmul)
        pss2 = a_ps.tile([1, NPC], FP, tag="pssa")
        psq2 = a_ps.tile([1, NPC], FP, tag="psqa")
        sq2 = ttmp.tile([128, 4, NPC], FP, tag="sq2")
        for kt in range(4):
            nc.vector.tensor_mul(out=sq2[:, kt, :], in0=op_sb[:, kt, :], in1=op_sb[:, kt, :])
        for kt in range(4):
            nc.tensor.matmul(pss2[:], ones_f[:], op_sb[:, kt, :],
                             start=(kt == 0), stop=(kt == 3))
            nc.tensor.matmul(psq2[:], ones_f[:], sq2[:, kt, :],
                             start=(kt == 0), stop=(kt == 3))
        mean2 = ttmp.tile([1, NPC], FP, tag="mean2")
        nc.vector.tensor_scalar(out=mean2[:], in0=pss2[:], scalar1=1.0 / E2, scalar2=None,
                                op0=ALU.mult)
        var2 = ttmp.tile([1, NPC], FP, tag="var2")
        nc.vector.tensor_mul(out=var2[:], in0=mean2[:], in1=mean2[:])
        sq_m = ttmp.tile([1, NPC], FP, tag="sq_m")
        nc.vector.tensor_scalar(out=sq_m[:], in0=psq2[:], scalar1=1.0 / E2, scalar2=None,
                                op0=ALU.mult)
        nc.vector.tensor_tensor(out=var2[:], in0=sq_m[:], in1=var2[:], op=ALU.subtract)
        mb3 = a_ps.tile([128, NPC], FP, tag="pssa")
        nc.tensor.matmul(mb3[:], ones_row[:], mean2[:], start=True, stop=True)
        vb3 = a_ps.tile([128, NPC], FP, tag="psqa")
        nc.tensor.matmul(vb3[:], ones_row[:], var2[:], start=True, stop=True)
        # rstd = exp(-0.5*ln(var+eps)) keeps us in the ln/exp ACT table set
        rstd3 = ttmp.tile([128, NPC], FP, tag="rstd3")
        nc.scalar.activation(out=rstd3[:], in_=vb3[:], func=AF.Ln, bias=eps_t[:],
                             scale=1.0)
        nc.scalar.activation(out=rstd3[:], in_=rstd3[:], func=AF.Exp, scale=-0.5)
        z_ln = tw.tile([128, 4, NPC], BF)
        for kt in range(4):
            cen2 = ttmp.tile([128, NPC], FP, tag="cen2")
            nc.vector.tensor_tensor(out=cen2[:], in0=op_sb[:, kt, :], in1=mb3[:],
                                    op=ALU.subtract)
            if flags["an1"]:
                nc.vector.tensor_tensor(out=z_ln[:, kt, :], in0=cen2[:], in1=rstd3[:],
                                        op=ALU.mult)
            else:
                zt2 = ttmp.tile([128, NPC], FP, tag="zt2")
                nc.vector.tensor_tensor(out=zt2[:], in0=cen2[:], in1=rstd3[:],
                                        op=ALU.mult)
                nc.vector.tensor_scalar(out=z_ln[:, kt, :], in0=zt2[:],
                                        scalar1=ang_sb[:, kt, :], scalar2=anb_sb[:, kt, :],
                                        op0=ALU.mult, op1=ALU.add)
        if dbg:
            for kt in range(4):
                dop = ttmp.tile([128, NPC], FP, tag="dop")
                nc.vector.tensor_copy(out=dop[:], in_=z_ln[:, kt, :])
                nc.sync.dma_start(out=dbg_opn[:, kt * NPC:(kt + 1) * NPC], in_=dop[:])
        # fuse head: [64, 512] then transpose to node-major output
        psf = p_ps.tile([64, NPC], FP, tag="psf")
        for kt in range(4):
            nc.tensor.matmul(psf[:], fw_sb[:, kt, :], z_ln[:, kt, :],
                             start=(kt == 0), stop=(kt == 3))
        ff_sb = ttmp.tile([64, NPC], FP, tag="ff_sb")
        if flags["fb0"]:
            nc.vector.tensor_copy(out=ff_sb[:], in_=psf[:])
        else:
            nc.vector.tensor_scalar(out=ff_sb[:], in0=psf[:], scalar1=fb_sb[:],
                                    scalar2=None, op0=ALU.add)
        for nt in range(NTC):
            ptf = a_ps.tile([128, 64], FP, tag="ptf")
            nc.tensor.matmul(ptf[:], ff_sb[:, nt * 128:(nt + 1) * 128], ident[:64, :64],
                             is_transpose=True)
            of_sb = ttmp.tile([128, 64], FP, tag="of_sb")
            nc.vector.tensor_copy(out=of_sb[:], in_=ptf[:])
            nc.sync.dma_start(out=out_d[nt * 128:(nt + 1) * 128, :], in_=of_sb[:])
        tailw.close()
        tl_ps.close()
        attn.close()
        cnn.close()
        stack.close()

    nc.compile()
    return nc


def _prep(inputs):
    """Host-side shard/transform. Returns per-core in_maps + flags."""
    x = np.ascontiguousarray(np.asarray(inputs["x"], np.float32))
    ei = np.asarray(inputs["edge_index"], np.int64)
    src, dst = ei[0], ei[1]
    deg = np.bincount(dst, minlength=N).astype(np.float32)
    deg = np.maximum(deg, 1.0)
    A = np.zeros((N, N), np.float32)
    np.add.at(A, (dst, src), 1.0)   # raw counts; 1/deg applied on-device

    def bfT(a, shape):
        return np.ascontiguousarray(a).astype(BF_NP).reshape(shape)

    sage_wl = np.asarray(inputs["sage_wl"], np.float32)
    sage_wr = np.asarray(inputs["sage_wr"], np.float32)
    wlT = bfT(sage_wl.transpose(0, 2, 1), (L, FT, 128, H))
    wrT = bfT(sage_wr.transpose(0, 2, 1), (L, FT, 128, H))
    conv_w = np.asarray(inputs["conv_w"], np.float32)       # [3, O, I, K]
    cwT = bfT(conv_w.transpose(0, 3, 2, 1), (3, 3, FT, 128, H))
    ipwT = bfT(np.asarray(inputs["in_proj_w"], np.float32).T, (4, 128, 3 * E2))
    opwT = bfT(np.asarray(inputs["out_proj_w"], np.float32).T, (4, 128, E2))
    fwT = bfT(np.asarray(inputs["fuse_w"], np.float32).T, (4, 128, OUT))

    bl = np.asarray(inputs["sage_bl"], np.float32)
    lng = np.asarray(inputs["ln_g"], np.float32)
    lnb = np.asarray(inputs["ln_b"], np.float32)
    cb = np.asarray(inputs["conv_b"], np.float32)
    cng = np.asarray(inputs["cnorm_g"], np.float32)
    cnb = np.asarray(inputs["cnorm_b"], np.float32)
    ipb = np.asarray(inputs["in_proj_b"], np.float32)
    opb = np.asarray(inputs["out_proj_b"], np.float32)
    ang = np.asarray(inputs["anorm_g"], np.float32)
    anb = np.asarray(inputs["anorm_b"], np.float32)
    fb = np.asarray(inputs["fuse_b"], np.float32)
    flags = {
        "bl0": bool(np.all(bl == 0)),
        "ln1": bool(np.all(lng == 1) and np.all(lnb == 0)),
        "cb0": bool(np.all(cb == 0)),
        "cn1": bool(np.all(cng == 1) and np.all(cnb == 0)),
        "ipb0": bool(np.all(ipb == 0)),
        "opb0": bool(np.all(opb == 0)),
        "an1": bool(np.all(ang == 1) and np.all(anb == 0)),
        "fb0": bool(np.all(fb == 0)),
    }

    x_bf = x.astype(BF_NP)
    in_maps = []
    for c in range(NCORES):
        rows = slice(c * NPC, (c + 1) * NPC)
        a_ct_c = np.ascontiguousarray(A[rows].T).astype(BF_NP).reshape(NT, 128, NPC)
        inv = (1.0 / deg[rows]).astype(np.float32)
        # halo indices into cc_halo_out [8 * 256, H]: rank r rows [256r, 256r+256)
        if c > 0:
            idx_l = (256 * (c - 1) + 128 + np.arange(HALO)).astype(np.int32)
        else:
            idx_l = np.zeros(HALO, np.int32)
        if c < NCORES - 1:
            idx_r = (256 * (c + 1) + np.arange(HALO)).astype(np.int32)
        else:
            idx_r = np.zeros(HALO, np.int32)
        m = {
            "x_bf": x_bf,
            "x_own": np.ascontiguousarray(x[rows]),
            "a_ct": a_ct_c,
            "invdeg": np.ascontiguousarray(inv.reshape(NTC, 128).T.reshape(128, NTC, 1)),
            "wlT": wlT, "wrT": wrT, "cwT": cwT,
            "ipwT": ipwT, "opwT": opwT, "fwT": fwT,
            "idx_l": idx_l.reshape(HALO, 1),
            "idx_r": idx_r.reshape(HALO, 1),
            "mask_l": np.full((HALO, 1), 0.0 if c == 0 else 1.0, np.float32),
            "mask_r": np.full((HALO, 1), 0.0 if c == NCORES - 1 else 1.0, np.float32),
        }
        if not flags["bl0"]:
            m["bl_in"] = np.ascontiguousarray(bl.reshape(1, L, H))
        if not flags["ln1"]:
            m["lng_in"] = np.ascontiguousarray(lng.reshape(1, L - 1, H))
            m["lnb_in"] = np.ascontiguousarray(lnb.reshape(1, L - 1, H))
        if not flags["cb0"]:
            m["cb_in"] = np.ascontiguousarray(
                cb.reshape(3, FT, 128).transpose(2, 0, 1).reshape(128, 3, FT, 1))
        if not flags["cn1"]:
            m["cng_in"] = np.ascontiguousarray(
                cng.reshape(3, FT, 128).transpose(2, 0, 1).reshape(128, 3, FT, 1))
            m["cnb_in"] = np.ascontiguousarray(
                cnb.reshape(3, FT, 128).transpose(2, 0, 1).reshape(128, 3, FT, 1))
        if not flags["ipb0"]:
            m["ipb_in"] = np.ascontiguousarray(
                ipb.reshape(12, 128).T.reshape(128, 12, 1))
        if not flags["opb0"]:
            m["opb_in"] = np.ascontiguousarray(
                opb.reshape(4, 128).T.reshape(128, 4, 1))
        if not flags["an1"]:
            m["ang_in"] = np.ascontiguousarray(
                ang.reshape(4, 128).T.reshape(128, 4, 1))
            m["anb_in"] = np.ascontiguousarray(
                anb.reshape(4, 128).T.reshape(128, 4, 1))
        if not flags["fb0"]:
            m["fb_in"] = np.ascontiguousarray(fb.reshape(OUT, 1))
        in_maps.append(m)
    return in_maps, flags


last_exec_time_ns = None
last_debug = None


def kernel(**inputs) -> np.ndarray:
    global last_exec_time_ns, last_debug
    import os
    dbg = os.environ.get("KERNEL_DEBUG", "0") == "1"
    in_maps, flags = _prep(inputs)
    key = (dbg,) + tuple(sorted(flags.items()))
    if key not in _CACHE:
        _CACHE[key] = _build(flags, dbg=dbg)
    nc = _CACHE[key]
    trace = os.environ.get("KERNEL_TRACE", "0") == "1"
    res = run_bass_kernel_spmd(nc, in_maps, core_ids=list(range(NCORES)),
                               trace=trace)
    last_exec_time_ns = res.exec_time_ns
    if dbg:
        last_debug = res.results
    out = np.concatenate([r["out"] for r in res.results], axis=0)
    return out.astype(np.float32)


# revision 20
# speedup vs baseline: 2.1171x; 1.0091x over previous
# Trainium2 Bass kernel for AdvancedHybridHOIGNN (6x SAGEConv + 3x Conv1d + MHA + fuse).
#
# Sharding: 4096 nodes split 512/core across 8 cores. The SAGE neighbor
# aggregation is dense matmuls against a host-built adjacency-count slice
# (bf16-exact counts; the 1/deg scaling is folded in after the wl matmul as a
# per-partition scale). Node features are replicated each layer via AllGather
# (bf16 payloads). The CNN branch needs a 128-node halo, exchanged with a
# small first/last-tile AllGather and fetched by indirect DMA with per-core
# index data so the single SPMD program stays uniform. Attention is
# sequence-parallel over queries with AllGathered K/V.
#
# fp32 matmuls execute as TWO passes on trn2 (FP32HI/LO), so all matmul
# operands are bf16; accumulation stays fp32 in PSUM, and LayerNorm/residual
# paths stay fp32.

import sys

sys.path.insert(0, "/opt/trn_rl_repo")

import ml_dtypes
import numpy as np

import concourse.bass as bass
import concourse.mybir as mybir
import concourse.tile as tile
from concourse import bacc
from concourse.bass_utils import run_bass_kernel_spmd
from concourse.masks import make_identity

FP = mybir.dt.float32
BF = mybir.dt.bfloat16
AF = mybir.ActivationFunctionType
ALU = mybir.AluOpType
BF_NP = ml_dtypes.bfloat16

N, H, OUT, L = 4096, 256, 64, 6
NCORES = 8
NPC = N // NCORES          # 512 nodes per core
NT = N // 128              # 32 node tiles globally
NTC = NPC // 128           # 4 node tiles per core
FT = H // 128              # 2 feature tiles
E2 = 2 * H                 # 512
HEADS, HD = 4, 128
HALO = 128                 # halo width for the conv branch (needs only 3)
CW = NPC + 2 * HALO        # 768: conv working width per core
EPS = 1e-5

_CACHE = {}


def _build(flags, dbg=False):
    nc = bacc.Bacc("TRN2", target_bir_lowering=False, debug=False,
                   num_devices=NCORES)
    RG = [list(range(NCORES))]

    # ---------------- kernel I/O ----------------
    x_bf = nc.dram_tensor("x_bf", [N, H], BF, kind="ExternalInput")
    x_own = nc.dram_tensor("x_own", [NPC, H], FP, kind="ExternalInput")
    a_ct = nc.dram_tensor("a_ct", [NT, 128, NPC], BF, kind="ExternalInput")
    invdeg = nc.dram_tensor("invdeg", [128, NTC, 1], FP, kind="ExternalInput")
    wlT = nc.dram_tensor("wlT", [L, FT, 128, H], BF, kind="ExternalInput")
    wrT = nc.dram_tensor("wrT", [L, FT, 128, H], BF, kind="ExternalInput")
    cwT = nc.dram_tensor("cwT", [3, 3, FT, 128, H], BF, kind="ExternalInput")
    ipwT = nc.dram_tensor("ipwT", [4, 128, 3 * E2], BF, kind="ExternalInput")
    opwT = nc.dram_tensor("opwT", [4, 128, E2], BF, kind="ExternalInput")
    fwT = nc.dram_tensor("fwT", [4, 128, OUT], BF, kind="ExternalInput")
    idx_l = nc.dram_tensor("idx_l", [HALO, 1], mybir.dt.int32, kind="ExternalInput")
    idx_r = nc.dram_tensor("idx_r", [HALO, 1], mybir.dt.int32, kind="ExternalInput")
    mask_l = nc.dram_tensor("mask_l", [HALO, 1], FP, kind="ExternalInput")
    mask_r = nc.dram_tensor("mask_r", [HALO, 1], FP, kind="ExternalInput")
    if not flags["bl0"]:
        bl_in = nc.dram_tensor("bl_in", [1, L, H], FP, kind="ExternalInput")
    if not flags["ln1"]:
        lng_in = nc.dram_tensor("lng_in", [1, L - 1, H], FP, kind="ExternalInput")
        lnb_in = nc.dram_tensor("lnb_in", [1, L - 1, H], FP, kind="ExternalInput")
    if not flags["cb0"]:
        cb_in = nc.dram_tensor("cb_in", [128, 3, FT, 1], FP, kind="ExternalInput")
    if not flags["cn1"]:
        cng_in = nc.dram_tensor("cng_in", [128, 3, FT, 1], FP, kind="ExternalInput")
        cnb_in = nc.dram_tensor("cnb_in", [128, 3, FT, 1], FP, kind="ExternalInput")
    if not flags["ipb0"]:
        ipb_in = nc.dram_tensor("ipb_in", [128, 12, 1], FP, kind="ExternalInput")
    if not flags["opb0"]:
        opb_in = nc.dram_tensor("opb_in", [128, 4, 1], FP, kind="ExternalInput")
    if not flags["an1"]:
        ang_in = nc.dram_tensor("ang_in", [128, 4, 1], FP, kind="ExternalInput")
        anb_in = nc.dram_tensor("anb_in", [128, 4, 1], FP, kind="ExternalInput")
    if not flags["fb0"]:
        fb_in = nc.dram_tensor("fb_in", [64, 1], FP, kind="ExternalInput")
    out_d = nc.dram_tensor("out", [NPC, OUT], FP, kind="ExternalOutput")
    if dbg:
        dbg_gnn = nc.dram_tensor("dbg_gnn", [NPC, H], FP, kind="ExternalOutput")
        dbg_cnn = nc.dram_tensor("dbg_cnn", [128, FT * CW], BF, kind="ExternalOutput")
        dbg_qkv = nc.dram_tensor("dbg_qkv", [128, 12 * NPC], BF, kind="ExternalOutput")
        dbg_ofm = nc.dram_tensor("dbg_ofm", [128, HEADS * NPC], BF, kind="ExternalOutput")
        dbg_opn = nc.dram_tensor("dbg_opn", [128, 4 * NPC], FP, kind="ExternalOutput")

    from contextlib import ExitStack

    with tile.TileContext(nc) as tc:
        stack = ExitStack()
        # long-lived pools on the right SBUF side; phase pools stack LIFO left
        singles = stack.enter_context(tc.tile_pool(name="singles", bufs=1, side="right"))
        dram = stack.enter_context(tc.tile_pool(name="dram", bufs=1, space="DRAM"))

        ident = singles.tile([128, 128], FP)
        make_identity(nc, ident[:])
        ident_bf = singles.tile([128, 128], BF)
        make_identity(nc, ident_bf[:])
        eps_t = singles.tile([128, 1], FP)
        nc.vector.memset(eps_t[:], EPS)
        ones_bf = singles.tile([128, 1], BF)
        nc.vector.memset(ones_bf[:], 1.0)
        ones_f = singles.tile([128, 1], FP)
        nc.vector.memset(ones_f[:], 1.0)
        ones_row = singles.tile([1, 128], FP)
        nc.vector.memset(ones_row[:], 1.0)
        invdeg_sb = singles.tile([128, NTC, 1], FP)
        nc.sync.dma_start(out=invdeg_sb[:], in_=invdeg[:])

        # ---- phase 1: SAGE layers ----
        sagew = ExitStack()
        sw = sagew.enter_context(tc.tile_pool(name="sagew", bufs=1))
        act_sb = sw.tile([128, NT, NPC], BF)          # adjacency counts, resident
        for s in range(NT):
            nc.sync.dma_start(out=act_sb[:, s, :], in_=a_ct[s])
        wl_sb = sw.tile([128, L, FT, H], BF)
        wr_sb = sw.tile([128, L, FT, H], BF)
        for i in range(L):
            for ft in range(FT):
                nc.sync.dma_start(out=wl_sb[:, i, ft, :], in_=wlT[i, ft])
                nc.sync.dma_start(out=wr_sb[:, i, ft, :], in_=wrT[i, ft])
        if not flags["bl0"]:
            bl_sb = sw.tile([128, L, H], FP)
            nc.gpsimd.dma_start(out=bl_sb[:], in_=bl_in[:].to_broadcast([128, L, H]))
        if not flags["ln1"]:
            lng_sb = sw.tile([128, L - 1, H], FP)
            lnb_sb = sw.tile([128, L - 1, H], FP)
            nc.gpsimd.dma_start(out=lng_sb[:], in_=lng_in[:].to_broadcast([128, L - 1, H]))
            nc.gpsimd.dma_start(out=lnb_sb[:], in_=lnb_in[:].to_broadcast([128, L - 1, H]))

        ho_pool = stack.enter_context(tc.tile_pool(name="ho", bufs=2, side="right"))
        hs_pool = ExitStack()
        hstream = hs_pool.enter_context(tc.tile_pool(name="hstream", bufs=8))
        sage_ps = ExitStack()
        agg_ps = sage_ps.enter_context(tc.tile_pool(name="agg_ps", bufs=1, space="PSUM"))
        z_ps = sage_ps.enter_context(tc.tile_pool(name="z_ps", bufs=2, space="PSUM"))
        t_ps = sage_ps.enter_context(tc.tile_pool(name="t_ps", bufs=2, space="PSUM"))
        sage_tmp = ExitStack()
        stmp = sage_tmp.enter_context(tc.tile_pool(name="stmp", bufs=6))
        aggp = sage_tmp.enter_context(tc.tile_pool(name="aggsb", bufs=2))

        # initial own-slice: node-major fp32 + feature-major bf16
        ho_nm = ho_pool.tile([128, NTC, H], FP, name="ho_nm0", tag="ho_nm")
        for nt in range(NTC):
            nc.sync.dma_start(out=ho_nm[:, nt, :], in_=x_own[nt * 128:(nt + 1) * 128, :])
        ho_fm = ho_pool.tile([128, FT, NPC], BF, name="ho_fm0", tag="ho_fm")
        for nt in range(NTC):
            for ft in range(FT):
                pt = t_ps.tile([128, 128], FP, tag="tps")
                nc.tensor.transpose(pt[:], ho_nm[:, nt, ft * 128:(ft + 1) * 128], ident[:])
                nc.vector.tensor_copy(out=ho_fm[:, ft, nt * 128:(nt + 1) * 128], in_=pt[:])

        cc_outs = []
        cc_halo_out = None
        for i in range(L):
            hsrc = x_bf if i == 0 else cc_outs[i - 1]
            # agg_cnt_fm[f, d] = sum_s h[s, f] * count[s, d]
            psa = [agg_ps.tile([128, NPC], FP, name=f"psa{i}_{ft}", tag=f"psa{ft}")
                   for ft in range(FT)]
            for s in range(NT):
                hk = hstream.tile([128, H], BF, tag="hk")
                nc.sync.dma_start(out=hk[:], in_=hsrc[s * 128:(s + 1) * 128, :])
                for ft in range(FT):
                    nc.tensor.matmul(psa[ft][:], hk[:, ft * 128:(ft + 1) * 128],
                                     act_sb[:, s, :], start=(s == 0), stop=(s == NT - 1))
            agg_fm = aggp.tile([128, FT, NPC], BF, tag="agg_fm")
            for ft in range(FT):
                nc.vector.tensor_copy(out=agg_fm[:, ft, :], in_=psa[ft][:])

            # z[n, o] = (agg_cnt @ wl.T) * invdeg + h @ wr.T   (node-major out)
            ho_nm_new = ho_pool.tile([128, NTC, H], FP, name=f"ho_nm{i + 1}", tag="ho_nm")
            ho_bf_new = ho_pool.tile([128, NTC, H], BF, name=f"ho_bf{i + 1}", tag="ho_bf")
            ho_fm_new = ho_pool.tile([128, FT, NPC], BF, name=f"ho_fm{i + 1}", tag="ho_fm")
            for nt in range(NTC):
                ns = slice(nt * 128, (nt + 1) * 128)
                psza = z_ps.tile([128, H], FP, tag="psza")
                for ft in range(FT):
                    nc.tensor.matmul(psza[:], agg_fm[:, ft, ns], wl_sb[:, i, ft, :],
                                     start=(ft == 0), stop=(ft == FT - 1))
                pszr = z_ps.tile([128, H], FP, tag="pszr")
                for ft in range(FT):
                    nc.tensor.matmul(pszr[:], ho_fm[:, ft, ns], wr_sb[:, i, ft, :],
                                     start=(ft == 0), stop=(ft == FT - 1))
                zt = stmp.tile([128, H], FP, tag="zt")
                nc.vector.tensor_scalar(out=zt[:], in0=psza[:],
                                        scalar1=invdeg_sb[:, nt, :], scalar2=None,
                                        op0=ALU.mult)
                z_sb = stmp.tile([128, H], FP, tag="z_sb")
                nc.vector.tensor_tensor(out=z_sb[:], in0=zt[:], in1=pszr[:], op=ALU.add)
                if not flags["bl0"]:
                    nc.vector.tensor_tensor(out=z_sb[:], in0=z_sb[:],
                                            in1=bl_sb[:, i, :], op=ALU.add)
                if i < L - 1:
                    stat = stmp.tile([128, 6], FP, tag="stat")
                    nc.vector.bn_stats(out=stat[:], in_=z_sb[:])
                    mv = stmp.tile([128, 2], FP, tag="mv")
                    nc.vector.bn_aggr(out=mv[:], in_=stat[:])
                    sd = stmp.tile([128, 1], FP, tag="sd")
                    nc.scalar.activation(out=sd[:], in_=mv[:, 1:2], func=AF.Sqrt,
                                         bias=eps_t[:], scale=1.0)
                    nc.vector.reciprocal(out=sd[:], in_=sd[:])
                    zn = stmp.tile([128, H], FP, tag="zn")
                    nc.vector.tensor_scalar(out=zn[:], in0=z_sb[:], scalar1=mv[:, 0:1],
                                            scalar2=sd[:], op0=ALU.subtract, op1=ALU.mult)
                    if not flags["ln1"]:
                        nc.vector.tensor_tensor(out=zn[:], in0=zn[:],
                                                in1=lng_sb[:, i, :], op=ALU.mult)
                        nc.vector.tensor_tensor(out=zn[:], in0=zn[:],
                                                in1=lnb_sb[:, i, :], op=ALU.add)
                else:
                    zn = z_sb
                zr = stmp.tile([128, H], FP, tag="zr")
                nc.scalar.activation(out=zr[:], in_=zn[:], func=AF.Relu)
                nc.vector.tensor_add(out=ho_nm_new[:, nt, :], in0=zr[:], in1=ho_nm[:, nt, :])
                nc.vector.tensor_copy(out=ho_bf_new[:, nt, :], in_=ho_nm_new[:, nt, :])
                for ft in range(FT):
                    pt = t_ps.tile([128, 128], FP, tag="tps")
                    nc.tensor.transpose(pt[:], ho_nm_new[:, nt, ft * 128:(ft + 1) * 128],
                                        ident[:])
                    nc.vector.tensor_copy(out=ho_fm_new[:, ft, ns], in_=pt[:])
            ho_nm, ho_fm, ho_bf = ho_nm_new, ho_fm_new, ho_bf_new

            if i < L - 1:
                cc_in = dram.tile([NPC, H], BF, name=f"cc_in{i}")
                for nt in range(NTC):
                    nc.sync.dma_start(out=cc_in[nt * 128:(nt + 1) * 128, :],
                                      in_=ho_bf[:, nt, :])
                cc_out = dram.tile([N, H], BF, name=f"cc_out{i}", addr_space="Shared")
                nc.gpsimd.collective_compute("AllGather", ALU.bypass, replica_groups=RG,
                                             ins=[cc_in.opt()], outs=[cc_out.opt()])
                cc_outs.append(cc_out)
            else:
                # last layer: only the conv halo needs neighbours
                cc_halo_in = dram.tile([2 * 128, H], BF, name="cc_halo_in")
                nc.sync.dma_start(out=cc_halo_in[0:128, :], in_=ho_bf[:, 0, :])
                nc.sync.dma_start(out=cc_halo_in[128:256, :], in_=ho_bf[:, NTC - 1, :])
                cc_halo_out = dram.tile([NCORES * 2 * 128, H], BF, name="cc_halo_out",
                                        addr_space="Shared")
                nc.gpsimd.collective_compute("AllGather", ALU.bypass, replica_groups=RG,
                                             ins=[cc_halo_in.opt()],
                                             outs=[cc_halo_out.opt()])

        sage_tmp.close()
        hs_pool.close()
        sagew.close()
        sage_ps.close()
        if dbg:
            for nt in range(NTC):
                nc.sync.dma_start(out=dbg_gnn[nt * 128:(nt + 1) * 128, :],
                                  in_=ho_nm[:, nt, :])

        # ---- phase 2: CNN branch (feature-major, nodes on free axis) ----
        cnn = ExitStack()
        cw_pool = cnn.enter_context(tc.tile_pool(name="cnnw", bufs=1))
        cfm_pool = cnn.enter_context(tc.tile_pool(name="cfm", bufs=2))
        ctmp = cnn.enter_context(tc.tile_pool(name="ctmp", bufs=1))
        cps = ExitStack()
        c_ps = cps.enter_context(tc.tile_pool(name="c_ps", bufs=2, space="PSUM"))
        s_ps = cps.enter_context(tc.tile_pool(name="s_ps", bufs=1, space="PSUM"))
        ctp_es = ExitStack()
        ct_ps = ctp_es.enter_context(tc.tile_pool(name="ct_ps", bufs=1, space="PSUM"))

        cw_sb = cw_pool.tile([128, 3, 3, FT, H], BF)
        for j in range(3):
            for k in range(3):
                for ft in range(FT):
                    nc.sync.dma_start(out=cw_sb[:, j, k, ft, :], in_=cwT[j, k, ft])
        if not flags["cb0"]:
            cb_sb = cw_pool.tile([128, 3, FT, 1], FP)
            nc.sync.dma_start(out=cb_sb[:], in_=cb_in[:])
        if not flags["cn1"]:
            cng_sb = cw_pool.tile([128, 3, FT, 1], FP)
            cnb_sb = cw_pool.tile([128, 3, FT, 1], FP)
            nc.sync.dma_start(out=cng_sb[:], in_=cng_in[:])
            nc.sync.dma_start(out=cnb_sb[:], in_=cnb_in[:])

        il_sb = cw_pool.tile([HALO, 1], mybir.dt.int32)
        ir_sb = cw_pool.tile([HALO, 1], mybir.dt.int32)
        ml_sb = cw_pool.tile([HALO, 1], FP)
        mr_sb = cw_pool.tile([HALO, 1], FP)
        nc.sync.dma_start(out=il_sb[:], in_=idx_l[:])
        nc.sync.dma_start(out=ir_sb[:], in_=idx_r[:])
        nc.sync.dma_start(out=ml_sb[:], in_=mask_l[:])
        nc.sync.dma_start(out=mr_sb[:], in_=mask_r[:])
        halo_l = ctmp.tile([HALO, H], BF, tag="halo")
        nc.gpsimd.indirect_dma_start(
            out=halo_l[:], out_offset=None, in_=cc_halo_out[:],
            in_offset=bass.IndirectOffsetOnAxis(ap=il_sb[:, :1], axis=0))
        nc.vector.tensor_scalar_mul(out=halo_l[:], in0=halo_l[:], scalar1=ml_sb[:])
        halo_r = ctmp.tile([HALO, H], BF, tag="halo")
        nc.gpsimd.indirect_dma_start(
            out=halo_r[:], out_offset=None, in_=cc_halo_out[:],
            in_offset=bass.IndirectOffsetOnAxis(ap=ir_sb[:, :1], axis=0))
        nc.vector.tensor_scalar_mul(out=halo_r[:], in0=halo_r[:], scalar1=mr_sb[:])

        c_fm = cfm_pool.tile([128, FT, CW], BF, tag="c_fm", name="c_fm_in")
        for w in range(6):
            for ft in range(FT):
                if w == 0 or w == 5:
                    hsrc2 = halo_l if w == 0 else halo_r
                    ptb = ct_ps.tile([128, 128], BF, tag="ctpsb")
                    nc.tensor.transpose(ptb[:], hsrc2[:, ft * 128:(ft + 1) * 128],
                                        ident_bf[:])
                    nc.vector.tensor_copy(out=c_fm[:, ft, w * 128:(w + 1) * 128],
                                          in_=ptb[:])
                else:
                    pt = ct_ps.tile([128, 128], FP, tag="ctps")
                    nc.tensor.transpose(pt[:], ho_nm[:, w - 1, ft * 128:(ft + 1) * 128],
                                        ident[:])
                    nc.vector.tensor_copy(out=c_fm[:, ft, w * 128:(w + 1) * 128],
                                          in_=pt[:])
        ctp_es.close()

        # conv layers: compute output cols [1, CW-1)
        chunks = [(1, 512), (513, CW - 1 - 513)]
        W = CW - 2
        for j in range(3):
            cr = cfm_pool.tile([128, FT, CW], BF, tag="c_fm", name=f"c_fm{j}")
            for ft in range(FT):  # guard stale edge cols
                nc.vector.memset(cr[:, ft, 0:1], 0.0)
                nc.vector.memset(cr[:, ft, CW - 1:CW], 0.0)
            for ot in range(FT):
                for (c0, cl) in chunks:
                    psc = c_ps.tile([128, 512], FP, tag="psc")
                    first = True
                    for k in range(3):
                        for it in range(FT):
                            nc.tensor.matmul(
                                psc[:, :cl],
                                cw_sb[:, j, k, it, ot * 128:(ot + 1) * 128],
                                c_fm[:, it, c0 + k - 1:c0 + k - 1 + cl],
                                start=first, stop=(k == 2 and it == FT - 1))
                            first = False
                    if flags["cb0"]:
                        nc.scalar.activation(out=cr[:, ot, c0:c0 + cl], in_=psc[:, :cl],
                                             func=AF.Relu)
                    else:
                        nc.scalar.activation(out=cr[:, ot, c0:c0 + cl], in_=psc[:, :cl],
                                             func=AF.Relu, bias=cb_sb[:, j, ot, :],
                                             scale=1.0)
            # channel LayerNorm per node (partition reduce via ones-matmul)
            sums = ctmp.tile([1, CW], FP, tag="sums")
            sumsq = ctmp.tile([1, CW], FP, tag="sumsq")
            sqt = ctmp.tile([128, FT, CW], BF, tag="sqt")
            for ft in range(FT):
                nc.vector.tensor_mul(out=sqt[:, ft, 1:1 + W], in0=cr[:, ft, 1:1 + W],
                                     in1=cr[:, ft, 1:1 + W])
            for (c0, cl) in chunks:
                pss = s_ps.tile([1, 512], FP, tag="pss")
                psq = s_ps.tile([1, 512], FP, tag="psq")
                for ft in range(FT):
                    nc.tensor.matmul(pss[:, :cl], ones_bf[:], cr[:, ft, c0:c0 + cl],
                                     start=(ft == 0), stop=(ft == FT - 1))
                    nc.tensor.matmul(psq[:, :cl], ones_bf[:], sqt[:, ft, c0:c0 + cl],
                                     start=(ft == 0), stop=(ft == FT - 1))
                nc.vector.tensor_copy(out=sums[:, c0:c0 + cl], in_=pss[:, :cl])
                nc.vector.tensor_copy(out=sumsq[:, c0:c0 + cl], in_=psq[:, :cl])
            mean = ctmp.tile([1, CW], FP, tag="mean")
            nc.vector.tensor_scalar(out=mean[:, 1:1 + W], in0=sums[:, 1:1 + W],
                                    scalar1=1.0 / H, scalar2=None, op0=ALU.mult)
            var = ctmp.tile([1, CW], FP, tag="var")
            nc.vector.tensor_mul(out=var[:, 1:1 + W], in0=mean[:, 1:1 + W],
                                 in1=mean[:, 1:1 + W])
            nc.vector.tensor_scalar(out=sumsq[:, 1:1 + W], in0=sumsq[:, 1:1 + W],
                                    scalar1=1.0 / H, scalar2=None, op0=ALU.mult)
            nc.vector.tensor_tensor(out=var[:, 1:1 + W], in0=sumsq[:, 1:1 + W],
                                    in1=var[:, 1:1 + W], op=ALU.subtract)
            # broadcast mean/var to all partitions, then rstd = 1/sqrt(var+eps)
            for (c0, cl) in chunks:
                mb = s_ps.tile([128, 512], FP, tag="mb")
                nc.tensor.matmul(mb[:, :cl], ones_row[:], mean[:, c0:c0 + cl],
                                 start=True, stop=True)
                vb = s_ps.tile([128, 512], FP, tag="vb")
                nc.tensor.matmul(vb[:, :cl], ones_row[:], var[:, c0:c0 + cl],
                                 start=True, stop=True)
                rstd = ctmp.tile([128, 512], FP, tag="rstd")
                nc.scalar.activation(out=rstd[:, :cl], in_=vb[:, :cl], func=AF.Sqrt,
                                     bias=eps_t[:], scale=1.0)
                nc.vector.reciprocal(out=rstd[:, :cl], in_=rstd[:, :cl])
                for ft in range(FT):
                    cen = ctmp.tile([128, 512], FP, tag="cen")
                    nc.vector.tensor_tensor(out=cen[:, :cl], in0=cr[:, ft, c0:c0 + cl],
                                            in1=mb[:, :cl], op=ALU.subtract)
                    nc.vector.tensor_tensor(out=cr[:, ft, c0:c0 + cl], in0=cen[:, :cl],
                                            in1=rstd[:, :cl], op=ALU.mult)
                    if not flags["cn1"]:
                        nc.vector.tensor_scalar(out=cr[:, ft, c0:c0 + cl],
                                                in0=cr[:, ft, c0:c0 + cl],
                                                scalar1=cng_sb[:, j, ft, :],
                                                scalar2=cnb_sb[:, j, ft, :],
                                                op0=ALU.mult, op1=ALU.add)
            # re-zero the out-of-graph halo (cores 0/7): the reference
            # zero-pads at every conv layer
            for ft in range(FT):
                nc.vector.tensor_scalar_mul(out=cr[:, ft, 0:HALO],
                                            in0=cr[:, ft, 0:HALO], scalar1=ml_sb[:])
                nc.vector.tensor_scalar_mul(out=cr[:, ft, CW - HALO:CW],
                                            in0=cr[:, ft, CW - HALO:CW], scalar1=mr_sb[:])
            c_fm = cr
        cps.close()
        if dbg:
            for ft in range(FT):
                nc.sync.dma_start(out=dbg_cnn[:, ft * CW:(ft + 1) * CW],
                                  in_=c_fm[:, ft, :])

        # ---- phase 3: fused projection + attention ----
        attn = ExitStack()
        aw = attn.enter_context(tc.tile_pool(name="attnw", bufs=1))
        q_sb = aw.tile([128, HEADS, NPC], BF)
        o_fm = aw.tile([128, HEADS, NPC], BF)

        qkvtmp = ExitStack()
        qtp = qkvtmp.enter_context(tc.tile_pool(name="qkvtmp", bufs=1))
        ipw_sb = qtp.tile([128, 4, 3 * E2], BF)
        for kt in range(4):
            nc.sync.dma_start(out=ipw_sb[:, kt, :], in_=ipwT[kt])
        kvt_sb = qtp.tile([128, 8, NPC], BF)
        if not flags["ipb0"]:
            ipb_sb = qtp.tile([128, 12, 1], FP)
            nc.sync.dma_start(out=ipb_sb[:], in_=ipb_in[:])
        v_nm = qtp.tile([128, NTC, E2], BF)

        qkv_ps = ExitStack()
        q_ps = qkv_ps.enter_context(tc.tile_pool(name="q_ps", bufs=3, space="PSUM"))
        qt_ps = qkv_ps.enter_context(tc.tile_pool(name="qt_ps", bufs=2, space="PSUM"))
        fused = [ho_fm[:, 0, :], ho_fm[:, 1, :],
                 c_fm[:, 0, HALO:HALO + NPC], c_fm[:, 1, HALO:HALO + NPC]]
        for ot in list(range(8, 12)) + list(range(4, 8)) + list(range(4)):
            psq = q_ps.tile([128, NPC], FP, tag="psq")
            for kt in range(4):
                nc.tensor.matmul(psq[:], ipw_sb[:, kt, ot * 128:(ot + 1) * 128],
                                 fused[kt], start=(kt == 0), stop=(kt == 3))
            dst = q_sb[:, ot, :] if ot < 4 else kvt_sb[:, ot - 4, :]
            if flags["ipb0"]:
                nc.vector.tensor_copy(out=dst, in_=psq[:])
            else:
                nc.vector.tensor_scalar(out=dst, in0=psq[:],
                                        scalar1=ipb_sb[:, ot, :], scalar2=None,
                                        op0=ALU.add)
            if ot == 11:
                # v tiles done: transpose feature-major -> node-major and fire
                # the v AllGather while k/q projections still run
                for nt in range(NTC):
                    for vt in range(4):
                        ptb = qt_ps.tile([128, 128], BF, tag="qtps")
                        nc.tensor.transpose(ptb[:],
                                            kvt_sb[:, 4 + vt, nt * 128:(nt + 1) * 128],
                                            ident_bf[:])
                        nc.vector.tensor_copy(out=v_nm[:, nt, vt * 128:(vt + 1) * 128],
                                              in_=ptb[:])
                v_in = dram.tile([NPC, E2], BF, name="v_in")
                for nt in range(NTC):
                    nc.sync.dma_start(out=v_in[nt * 128:(nt + 1) * 128, :],
                                      in_=v_nm[:, nt, :])
                v_out = dram.tile([N, E2], BF, name="v_out", addr_space="Shared")
                nc.gpsimd.collective_compute("AllGather", ALU.bypass, replica_groups=RG,
                                             ins=[v_in.opt()], outs=[v_out.opt()])
            if ot == 7:
                k_in = dram.tile([E2, NPC], BF, name="k_in")
                for kt in range(4):
                    nc.sync.dma_start(out=k_in[kt * 128:(kt + 1) * 128, :],
                                      in_=kvt_sb[:, kt, :])
                k_out = dram.tile([NCORES * E2, NPC], BF, name="k_out",
                                  addr_space="Shared")
                nc.gpsimd.collective_compute("AllGather", ALU.bypass, replica_groups=RG,
                                             ins=[k_in.opt()], outs=[k_out.opt()])
        qkv_ps.close()
        if dbg:
            for ot in range(12):
                srcq = q_sb[:, ot, :] if ot < 4 else kvt_sb[:, ot - 4, :]
                nc.sync.dma_start(out=dbg_qkv[:, ot * NPC:(ot + 1) * NPC], in_=srcq)
        qkvtmp.close()

        heads_es = ExitStack()
        kv_pool = heads_es.enter_context(tc.tile_pool(name="kv", bufs=2))
        pt_pool = heads_es.enter_context(tc.tile_pool(name="ptp", bufs=2))
        at_ps = ExitStack()
        st_ps = at_ps.enter_context(tc.tile_pool(name="st_ps", bufs=2, space="PSUM"))
        o_ps = at_ps.enter_context(tc.tile_pool(name="o_ps", bufs=1, space="PSUM"))
        rs_ps = at_ps.enter_context(tc.tile_pool(name="rs_ps", bufs=1, space="PSUM"))
        inv_sqrt_hd = 1.0 / float(np.sqrt(HD))
        GRP = 3
        groups = [list(range(g, min(g + GRP, NT))) for g in range(0, NT, GRP)]
        for h in range(HEADS):
            k_sb = kv_pool.tile([128, N], BF, tag="k_sb")
            for r in range(NCORES):
                nc.sync.dma_start(out=k_sb[:, r * NPC:(r + 1) * NPC],
                                  in_=k_out[r * E2 + h * 128:r * E2 + (h + 1) * 128, :])
            v_sb = kv_pool.tile([128, NT, 128], BF, tag="v_sb")
            for st in range(NT):
                nc.sync.dma_start(out=v_sb[:, st, :],
                                  in_=v_out[st * 128:(st + 1) * 128,
                                            h * 128:(h + 1) * 128])
            pso = o_ps.tile([128, NPC], FP, tag="pso")
            acc = pt_pool.tile([128, NPC], FP, tag="acc")
            for grp in groups:
                pst = st_ps.tile([128, GRP * NPC], FP, tag="pst")
                for gi, s in enumerate(grp):
                    nc.tensor.matmul(pst[:, gi * NPC:(gi + 1) * NPC],
                                     k_sb[:, s * 128:(s + 1) * 128],
                                     q_sb[:, h, :], start=True, stop=True)
                p_sb = pt_pool.tile([128, GRP * NPC], BF, tag="p_sb")
                nc.scalar.activation(out=p_sb[:, :len(grp) * NPC],
                                     in_=pst[:, :len(grp) * NPC],
                                     func=AF.Exp, scale=inv_sqrt_hd)
                for gi, s in enumerate(grp):
                    rhs = p_sb[:, gi * NPC:(gi + 1) * NPC]
                    nc.tensor.matmul(pso[:], v_sb[:, s, :], rhs,
                                     start=(s == 0), stop=(s == NT - 1))
                    # rowsum accumulates on DVE instead of burning PE cycles
                    if s == 0:
                        nc.vector.tensor_copy(out=acc[:], in_=rhs)
                    else:
                        nc.vector.tensor_tensor(out=acc[:], in0=acc[:], in1=rhs,
                                                op=ALU.add)
            psr = rs_ps.tile([1, NPC], FP, tag="psr")
            nc.tensor.matmul(psr[:], ones_f[:], acc[:], start=True, stop=True)
            rs_sb = pt_pool.tile([1, NPC], FP, tag="rs_sb")
            nc.vector.tensor_copy(out=rs_sb[:], in_=psr[:])
            rb2 = rs_ps.tile([128, NPC], FP, tag="psr")
            nc.tensor.matmul(rb2[:], ones_row[:], rs_sb[:], start=True, stop=True)
            rr2 = pt_pool.tile([128, NPC], FP, tag="rr2")
            nc.vector.reciprocal(out=rr2[:], in_=rb2[:])
            nc.vector.tensor_tensor(out=o_fm[:, h, :], in0=rr2[:], in1=pso[:],
                                    op=ALU.mult)
        at_ps.close()
        heads_es.close()
        if dbg:
            for h in range(HEADS):
                nc.sync.dma_start(out=dbg_ofm[:, h * NPC:(h + 1) * NPC],
                                  in_=o_fm[:, h, :])

        # ---- phase 4: out_proj + layernorm + fuse head ----
        tailw = ExitStack()
        tw = tailw.enter_context(tc.tile_pool(name="tailw", bufs=1))
        ttmp = tailw.enter_context(tc.tile_pool(name="ttmp", bufs=1))
        tl_ps = ExitStack()
        p_ps = tl_ps.enter_context(tc.tile_pool(name="p_ps", bufs=2, space="PSUM"))
        a_ps = tl_ps.enter_context(tc.tile_pool(name="a_ps", bufs=1, space="PSUM"))
        opw_sb = tw.tile([128, 4, E2], BF)
        for kt in range(4):
            nc.sync.dma_start(out=opw_sb[:, kt, :], in_=opwT[kt])
        fw_sb = tw.tile([128, 4, OUT], BF)
        for kt in range(4):
            nc.sync.dma_start(out=fw_sb[:, kt, :], in_=fwT[kt])
        if not flags["opb0"]:
            opb_sb = tw.tile([128, 4, 1], FP)
            nc.sync.dma_start(out=opb_sb[:], in_=opb_in[:])
        if not flags["an1"]:
            ang_sb = tw.tile([128, 4, 1], FP)
            anb_sb = tw.tile([128, 4, 1], FP)
            nc.sync.dma_start(out=ang_sb[:], in_=ang_in[:])
            nc.sync.dma_start(out=anb_sb[:], in_=anb_in[:])
        if not flags["fb0"]:
            fb_sb = tw.tile([64, 1], FP)
            nc.sync.dma_start(out=fb_sb[:], in_=fb_in[:])

        op_sb = tw.tile([128, 4, NPC], FP)
        for ot in range(4):
            psp = p_ps.tile([128, NPC], FP, tag="psp")
            for kt in range(4):
                nc.tensor.matmul(psp[:], opw_sb[:, kt, ot * 128:(ot + 1) * 128],
                                 o_fm[:, kt, :], start=(kt == 0), stop=(kt == 3))
            if flags["opb0"]:
                nc.vector.tensor_copy(out=op_sb[:, ot, :], in_=psp[:])
            else:
                nc.vector.tensor_scalar(out=op_sb[:, ot, :], in0=psp[:],
                                        scalar1=opb_sb[:, ot, :], scalar2=None,
                                        op0=ALU.add)
        # anorm LN over 512 features (partition reduce via ones-matmul)
        pss2 = a_ps.tile([1, NPC], FP, tag="pssa")
        psq2 = a_ps.tile([1, NPC], FP, tag="psqa")
        sq2 = ttmp.tile([128, 4, NPC], FP, tag="sq2")
        for kt in range(4):
            nc.vector.tensor_mul(out=sq2[:, kt, :], in0=op_sb[:, kt, :], in1=op_sb[:, kt, :])
        for kt in range(4):
            nc.tensor.matmul(pss2[:], ones_f[:], op_sb[:, kt, :],
                             start=(kt == 0), stop=(kt == 3))
            nc.tensor.matmul(psq2[:], ones_f[:], sq2[:, kt, :],
                             start=(kt == 0), stop=(kt == 3))
        mean2 = ttmp.tile([1, NPC], FP, tag="mean2")
        nc.vector.tensor_scalar(out=mean2[:], in0=pss2[:], scalar1=1.0 / E2, scalar2=None,
                                op0=ALU.mult)
        var2 = ttmp.tile([1, NPC], FP, tag="var2")
        nc.vector.tensor_mul(out=var2[:], in0=mean2[:], in1=mean2[:])
        sq_m = ttmp.tile([1, NPC], FP, tag="sq_m")
        nc.vector.tensor_scalar(out=sq_m[:], in0=psq2[:], scalar1=1.0 / E2, scalar2=None,
                                op0=ALU.mult)
        nc.vector.tensor_tensor(out=var2[:], in0=sq_m[:], in1=var2[:], op=ALU.subtract)
        mb3 = a_ps.tile([128, NPC], FP, tag="pssa")
        nc.tensor.matmul(mb3[:], ones_row[:], mean2[:], start=True, stop=True)
        vb3 = a_ps.tile([128, NPC], FP, tag="psqa")
        nc.tensor.matmul(vb3[:], ones_row[:], var2[:], start=True, stop=True)
        # rstd = exp(-0.5*ln(var+eps)) keeps us in the ln/exp ACT table set
        rstd3 = ttmp.tile([128, NPC], FP, tag="rstd3")
        nc.scalar.activation(out=rstd3[:], in_=vb3[:], func=AF.Ln, bias=eps_t[:],
                             scale=1.0)
        nc.scalar.activation(out=rstd3[:], in_=rstd3[:], func=AF.Exp, scale=-0.5)
        z_ln = tw.tile([128, 4, NPC], BF)
        for kt in range(4):
            cen2 = ttmp.tile([128, NPC], FP, tag="cen2")
            nc.vector.tensor_tensor(out=cen2[:], in0=op_sb[:, kt, :], in1=mb3[:],
                                    op=ALU.subtract)
            if flags["an1"]:
                nc.vector.tensor_tensor(out=z_ln[:, kt, :], in0=cen2[:], in1=rstd3[:],
                                        op=ALU.mult)
            else:
                zt2 = ttmp.tile([128, NPC], FP, tag="zt2")
                nc.vector.tensor_tensor(out=zt2[:], in0=cen2[:], in1=rstd3[:],
                                        op=ALU.mult)
                nc.vector.tensor_scalar(out=z_ln[:, kt, :], in0=zt2[:],
                                        scalar1=ang_sb[:, kt, :], scalar2=anb_sb[:, kt, :],
                                        op0=ALU.mult, op1=ALU.add)
        if dbg:
            for kt in range(4):
                dop = ttmp.tile([128, NPC], FP, tag="dop")
                nc.vector.tensor_copy(out=dop[:], in_=z_ln[:, kt, :])
                nc.sync.dma_start(out=dbg_opn[:, kt * NPC:(kt + 1) * NPC], in_=dop[:])
        # fuse head: [64, 512] then transpose to node-major output
        psf = p_ps.tile([64, NPC], FP, tag="psf")
        for kt in range(4):
            nc.tensor.matmul(psf[:], fw_sb[:, kt, :], z_ln[:, kt, :],
                             start=(kt == 0), stop=(kt == 3))
        ff_sb = ttmp.tile([64, NPC], FP, tag="ff_sb")
        if flags["fb0"]:
            nc.vector.tensor_copy(out=ff_sb[:], in_=psf[:])
        else:
            nc.vector.tensor_scalar(out=ff_sb[:], in0=psf[:], scalar1=fb_sb[:],
                                    scalar2=None, op0=ALU.add)
        for nt in range(NTC):
            ptf = a_ps.tile([128, 64], FP, tag="ptf")
            nc.tensor.matmul(ptf[:], ff_sb[:, nt * 128:(nt + 1) * 128], ident[:64, :64],
                             is_transpose=True)
            of_sb = ttmp.tile([128, 64], FP, tag="of_sb")
            nc.vector.tensor_copy(out=of_sb[:], in_=ptf[:])
            nc.sync.dma_start(out=out_d[nt * 128:(nt + 1) * 128, :], in_=of_sb[:])
        tailw.close()
        tl_ps.close()
        attn.close()
        cnn.close()
        stack.close()

    nc.compile()
    return nc


def _prep(inputs):
    """Host-side shard/transform. Returns per-core in_maps + flags."""
    x = np.ascontiguousarray(np.asarray(inputs["x"], np.float32))
    ei = np.asarray(inputs["edge_index"], np.int64)
    src, dst = ei[0], ei[1]
    deg = np.bincount(dst, minlength=N).astype(np.float32)
    deg = np.maximum(deg, 1.0)
    A = np.zeros((N, N), np.float32)
    np.add.at(A, (dst, src), 1.0)   # raw counts; 1/deg applied on-device

    def bfT(a, shape):
        return np.ascontiguousarray(a).astype(BF_NP).reshape(shape)

    sage_wl = np.asarray(inputs["sage_wl"], np.float32)
    sage_wr = np.asarray(inputs["sage_wr"], np.float32)
    wlT = bfT(sage_wl.transpose(0, 2, 1), (L, FT, 128, H))
    wrT = bfT(sage_wr.transpose(0, 2, 1), (L, FT, 128, H))
    conv_w = np.asarray(inputs["conv_w"], np.float32)       # [3, O, I, K]
    cwT = bfT(conv_w.transpose(0, 3, 2, 1), (3, 3, FT, 128, H))
    ipwT = bfT(np.asarray(inputs["in_proj_w"], np.float32).T, (4, 128, 3 * E2))
    opwT = bfT(np.asarray(inputs["out_proj_w"], np.float32).T, (4, 128, E2))
    fwT = bfT(np.asarray(inputs["fuse_w"], np.float32).T, (4, 128, OUT))

    bl = np.asarray(inputs["sage_bl"], np.float32)
    lng = np.asarray(inputs["ln_g"], np.float32)
    lnb = np.asarray(inputs["ln_b"], np.float32)
    cb = np.asarray(inputs["conv_b"], np.float32)
    cng = np.asarray(inputs["cnorm_g"], np.float32)
    cnb = np.asarray(inputs["cnorm_b"], np.float32)
    ipb = np.asarray(inputs["in_proj_b"], np.float32)
    opb = np.asarray(inputs["out_proj_b"], np.float32)
    ang = np.asarray(inputs["anorm_g"], np.float32)
    anb = np.asarray(inputs["anorm_b"], np.float32)
    fb = np.asarray(inputs["fuse_b"], np.float32)
    flags = {
        "bl0": bool(np.all(bl == 0)),
        "ln1": bool(np.all(lng == 1) and np.all(lnb == 0)),
        "cb0": bool(np.all(cb == 0)),
        "cn1": bool(np.all(cng == 1) and np.all(cnb == 0)),
        "ipb0": bool(np.all(ipb == 0)),
        "opb0": bool(np.all(opb == 0)),
        "an1": bool(np.all(ang == 1) and np.all(anb == 0)),
        "fb0": bool(np.all(fb == 0)),
    }

    x_bf = x.astype(BF_NP)
    in_maps = []
    for c in range(NCORES):
        rows = slice(c * NPC, (c + 1) * NPC)
        a_ct_c = np.ascontiguousarray(A[rows].T).astype(BF_NP).reshape(NT, 128, NPC)
        inv = (1.0 / deg[rows]).astype(np.float32)
        # halo indices into cc_halo_out [8 * 256, H]: rank r rows [256r, 256r+256)
        if c > 0:
            idx_l = (256 * (c - 1) + 128 + np.arange(HALO)).astype(np.int32)
        else:
            idx_l = np.zeros(HALO, np.int32)
        if c < NCORES - 1:
            idx_r = (256 * (c + 1) + np.arange(HALO)).astype(np.int32)
        else:
            idx_r = np.zeros(HALO, np.int32)
        m = {
            "x_bf": x_bf,
            "x_own": np.ascontiguousarray(x[rows]),
            "a_ct": a_ct_c,
            "invdeg": np.ascontiguousarray(inv.reshape(NTC, 128).T.reshape(128, NTC, 1)),
            "wlT": wlT, "wrT": wrT, "cwT": cwT,
            "ipwT": ipwT, "opwT": opwT, "fwT": fwT,
            "idx_l": idx_l.reshape(HALO, 1),
            "idx_r": idx_r.reshape(HALO, 1),
            "mask_l": np.full((HALO, 1), 0.0 if c == 0 else 1.0, np.float32),
            "mask_r": np.full((HALO, 1), 0.0 if c == NCORES - 1 else 1.0, np.float32),
        }
        if not flags["bl0"]:
            m["bl_in"] = np.ascontiguousarray(bl.reshape(1, L, H))
        if not flags["ln1"]:
            m["lng_in"] = np.ascontiguousarray(lng.reshape(1, L - 1, H))
            m["lnb_in"] = np.ascontiguousarray(lnb.reshape(1, L - 1, H))
        if not flags["cb0"]:
            m["cb_in"] = np.ascontiguousarray(
                cb.reshape(3, FT, 128).transpose(2, 0, 1).reshape(128, 3, FT, 1))
        if not flags["cn1"]:
            m["cng_in"] = np.ascontiguousarray(
                cng.reshape(3, FT, 128).transpose(2, 0, 1).reshape(128, 3, FT, 1))
            m["cnb_in"] = np.ascontiguousarray(
                cnb.reshape(3, FT, 128).transpose(2, 0, 1).reshape(128, 3, FT, 1))
        if not flags["ipb0"]:
            m["ipb_in"] = np.ascontiguousarray(
                ipb.reshape(12, 128).T.reshape(128, 12, 1))
        if not flags["opb0"]:
            m["opb_in"] = np.ascontiguousarray(
                opb.reshape(4, 128).T.reshape(128, 4, 1))
        if not flags["an1"]:
            m["ang_in"] = np.ascontiguousarray(
                ang.reshape(4, 128).T.reshape(128, 4, 1))
            m["anb_in"] = np.ascontiguousarray(
                anb.reshape(4, 128).T.reshape(128, 4, 1))
        if not flags["fb0"]:
            m["fb_in"] = np.ascontiguousarray(fb.reshape(OUT, 1))
        in_maps.append(m)
    return in_maps, flags


last_exec_time_ns = None
last_debug = None


def kernel(**inputs) -> np.ndarray:
    global last_exec_time_ns, last_debug
    import os
    dbg = os.environ.get("KERNEL_DEBUG", "0") == "1"
    in_maps, flags = _prep(inputs)
    key = (dbg,) + tuple(sorted(flags.items()))
    if key not in _CACHE:
        _CACHE[key] = _build(flags, dbg=dbg)
    nc = _CACHE[key]
    trace = os.environ.get("KERNEL_TRACE", "0") == "1"
    res = run_bass_kernel_spmd(nc, in_maps, core_ids=list(range(NCORES)),
                               trace=trace)
    last_exec_time_ns = res.exec_time_ns
    if dbg:
        last_debug = res.results
    out = np.concatenate([r["out"] for r in res.results], axis=0)
    return out.astype(np.float32)
